# revision 7
# baseline (speedup 1.0000x reference)
"""GNN message-passing kernel for 8 TRN2 NeuronCores.

Reference: 3 layers of (GCN + 4-head graph attention) over a 4096x4096
adjacency, plus encoder / classifier / contagion heads.

Algebraic reformulation of the attention (validated to ~1e-6 vs the
reference): non-edges get score exactly 0 (exp=1), so with u = exp(t),
es_i = exp(s_i + ab):

    probs @ hh = [es * (adj @ (u*hh)) + (S_all - adj @ hh)] / Z
    Z          = es * (adj @ u) + (N - deg)

Everything reduces to row-blocks of adj @ M where M is [N, 388]:
    cols [0:128)    dinv * h          (GCN, symmetric-normalized)
    cols [128:256)  u * hh            (4 heads x 32)
    cols [256:384)  -hh               (negated; ones x S_all added on top)
    cols [384:388)  u                 (4 heads)

Sharding: core c owns rows [512c, 512c+512). Its stationary operand is
adj[rows_c, :].T laid out k-major in SBUF as bf16 (adj is 0/1 -> exact).
M is split hi/lo into two bf16 matmuls accumulated in the same PSUM bank
(~1.5e-5 relative precision). h (N x 128) is all-gathered each layer.
"""

import numpy as np

NCORES = 8
N = 4096
F = 64
H = 128
NH = 4
HD = 32
L = 3
CDIM = 7
R = N // NCORES          # 512 rows per core
NT = N // 128            # 32 j-tiles
RT = R // 128            # 4 r-tiles per core
MW = H + H + H + NH      # 388 columns of M
USE_SPLIT = False        # bf16-only M measured at 4.6e-4 rel err (gate 2e-2)

_CACHE = {}


def _build_nc():
    import concourse.bass as bass
    import concourse.mybir as mybir
    from concourse import bacc
    import concourse.tile as tile
    from concourse.masks import make_identity

    dt = mybir.dt
    f32 = dt.float32
    bf16 = dt.bfloat16
    AF = mybir.ActivationFunctionType

    nc = bacc.Bacc("TRN2", target_bir_lowering=False, debug=False,
                   num_devices=NCORES)

    # ---- I/O ----
    adjT_d = nc.dram_tensor("adjT", [128, NT * R], f32, kind="ExternalInput").ap()
    xT_d = nc.dram_tensor("xT", [F, N], f32, kind="ExternalInput").ap()
    xTl_d = nc.dram_tensor("xT_loc", [F, R], f32, kind="ExternalInput").ap()
    encW_d = nc.dram_tensor("enc_W", [F, H], f32, kind="ExternalInput").ap()
    encb_d = nc.dram_tensor("enc_b", [H], f32, kind="ExternalInput").ap()
    gcnW_d = nc.dram_tensor("gcn_W", [L, H, H], f32, kind="ExternalInput").ap()
    gcnb_d = nc.dram_tensor("gcn_b", [L, H], f32, kind="ExternalInput").ap()
    awcat_d = nc.dram_tensor("attn_Wcat", [L, H, H], f32, kind="ExternalInput").ap()
    awt_d = nc.dram_tensor("attn_WT", [L, NH, HD, H], f32, kind="ExternalInput").ap()
    awb_d = nc.dram_tensor("attn_Wb", [L, NH, HD], f32, kind="ExternalInput").ap()
    aa_d = nc.dram_tensor("attn_a", [L, NH, 2 * HD], f32, kind="ExternalInput").ap()
    aab_d = nc.dram_tensor("attn_ab", [L, NH], f32, kind="ExternalInput").ap()
    cW1_d = nc.dram_tensor("cls_W1", [H, H // 2], f32, kind="ExternalInput").ap()
    cb1_d = nc.dram_tensor("cls_b1", [H // 2], f32, kind="ExternalInput").ap()
    cW2_d = nc.dram_tensor("cls_W2", [H // 2, CDIM], f32, kind="ExternalInput").ap()
    cb2_d = nc.dram_tensor("cls_b2", [CDIM], f32, kind="ExternalInput").ap()
    nW1_d = nc.dram_tensor("con_W1", [H, H // 2], f32, kind="ExternalInput").ap()
    nb1_d = nc.dram_tensor("con_b1", [H // 2], f32, kind="ExternalInput").ap()
    nW2_d = nc.dram_tensor("con_W2", [H // 2, 1], f32, kind="ExternalInput").ap()
    nb2_d = nc.dram_tensor("con_b2", [1], f32, kind="ExternalInput").ap()

    logT_d = nc.dram_tensor("logitsT", [CDIM, R], f32, kind="ExternalOutput").ap()
    outh_d = nc.dram_tensor("out_h", [R, H], f32, kind="ExternalOutput").ap()
    outc_d = nc.dram_tensor("out_con", [1, 1], f32, kind="ExternalOutput").ap()

    mult = mybir.AluOpType.mult
    add = mybir.AluOpType.add

    with tile.TileContext(nc) as tc:
        with (
            tc.tile_pool(name="persist", bufs=1) as pp,
            tc.tile_pool(name="chunk", bufs=2) as chp,
            tc.tile_pool(name="mpool", bufs=3) as mp,
            tc.tile_pool(name="work", bufs=3) as wp,
            tc.tile_pool(name="ps_y", bufs=4, space="PSUM") as psy,
            tc.tile_pool(name="ps_s", bufs=4, space="PSUM") as pss,
            tc.tile_pool(name="dram", bufs=1, space="DRAM") as dp,
        ):
            # ---------- constants ----------
            ident = pp.tile([128, 128], f32)
            make_identity(nc, ident)
            identb = pp.tile([128, 128], bf16)
            nc.vector.tensor_copy(identb[:], ident[:])
            ones_row = pp.tile([1, 128], bf16)
            nc.vector.memset(ones_row[:], 1.0)
            ones_rowf = pp.tile([1, 128], f32)
            nc.vector.memset(ones_rowf[:], 1.0)
            ncell = pp.tile([1, 1], f32)
            nc.vector.memset(ncell[:], float(N))
            ones_colb = pp.tile([128, 1], bf16)
            nc.vector.memset(ones_colb[:], 1.0)
            ones_colf = pp.tile([128, 1], f32)
            nc.vector.memset(ones_colf[:], 1.0)

            # ---------- small weights ----------
            encWf = pp.tile([F, H], f32)
            nc.sync.dma_start(out=encWf[:], in_=encW_d)
            encW = pp.tile([F, H], bf16)
            nc.vector.tensor_copy(encW[:], encWf[:])
            encbf = pp.tile([1, H], f32)
            nc.sync.dma_start(out=encbf[:], in_=encb_d.unsqueeze(0))
            encb = pp.tile([1, H], bf16)
            nc.vector.tensor_copy(encb[:], encbf[:])
            gcnW = pp.tile([128, L * H], f32)
            gcnbf = pp.tile([1, L * H], f32)
            for l in range(L):
                nc.sync.dma_start(out=gcnW[:, l * H:(l + 1) * H], in_=gcnW_d[l])
                nc.sync.dma_start(out=gcnbf[:, l * H:(l + 1) * H],
                                  in_=gcnb_d[l].unsqueeze(0))
            gcnb = pp.tile([1, L * H], bf16)
            nc.vector.tensor_copy(gcnb[:], gcnbf[:])
            cW1 = pp.tile([H, H // 2], f32)
            nc.sync.dma_start(out=cW1[:], in_=cW1_d)
            cb1 = pp.tile([H // 2, 1], f32)
            nc.sync.dma_start(out=cb1[:], in_=cb1_d.unsqueeze(1))
            cW2 = pp.tile([H // 2, CDIM], f32)
            nc.sync.dma_start(out=cW2[:], in_=cW2_d)
            cb2 = pp.tile([CDIM, 1], f32)
            nc.sync.dma_start(out=cb2[:], in_=cb2_d.unsqueeze(1))
            nW1 = pp.tile([H, H // 2], f32)
            nc.sync.dma_start(out=nW1[:], in_=nW1_d)
            nb1 = pp.tile([H // 2, 1], f32)
            nc.sync.dma_start(out=nb1[:], in_=nb1_d.unsqueeze(1))
            nW2 = pp.tile([H // 2, 1], f32)
            nc.sync.dma_start(out=nW2[:], in_=nW2_d)
            nb2 = pp.tile([1, 1], f32)
            nc.sync.dma_start(out=nb2[:], in_=nb2_d.unsqueeze(0))
            ab_sb = pp.tile([1, L * NH], f32)
            nc.sync.dma_start(out=ab_sb[:], in_=aab_d.flatten().unsqueeze(0))

            # per-(l,h) columns used to fold s/t into the hh matmul
            awt_sb = pp.tile([HD, L * NH * H], f32)
            adcol = pp.tile([HD, L * NH], f32)   # a_dst columns
            ascol = pp.tile([HD, L * NH], f32)   # a_src columns
            wbcol = pp.tile([HD, L * NH], f32)   # attn_Wb columns
            for l in range(L):
                for hh_ in range(NH):
                    k = l * NH + hh_
                    nc.sync.dma_start(out=awt_sb[:, k * H:(k + 1) * H],
                                      in_=awt_d[l, hh_])
                    nc.sync.dma_start(out=adcol[:, k:k + 1],
                                      in_=aa_d[l, hh_, HD:].unsqueeze(1))
                    nc.sync.dma_start(out=ascol[:, k:k + 1],
                                      in_=aa_d[l, hh_, 0:HD].unsqueeze(1))
                    nc.sync.dma_start(out=wbcol[:, k:k + 1],
                                      in_=awb_d[l, hh_].unsqueeze(1))

            # W_big[l] = [Wcat (128) | w_t (4) | w_s (4)]  and consts rows
            WB = H + 2 * NH  # 136
            wbig = pp.tile([128, L * WB], f32)
            wbigb = pp.tile([128, L * WB], bf16)
            consts = pp.tile([1, L * WB], f32)
            constsb = pp.tile([1, L * WB], bf16)
            for l in range(L):
                nc.sync.dma_start(out=wbig[:, l * WB:l * WB + H], in_=awcat_d[l])
                nc.sync.dma_start(out=consts[:, l * WB:l * WB + H],
                                  in_=awb_d[l].flatten().unsqueeze(0))
                for hh_ in range(NH):
                    k = l * NH + hh_
                    wt_ps = pss.tile([128, 1], f32, tag="ps_small", name=f"wt_{l}_{hh_}")
                    nc.tensor.matmul(wt_ps[:], awt_sb[:, k * H:(k + 1) * H],
                                     adcol[:, k:k + 1], start=True, stop=True)
                    nc.vector.tensor_copy(wbig[:, l * WB + H + hh_:l * WB + H + hh_ + 1],
                                          wt_ps[:])
                    ws_ps = pss.tile([128, 1], f32, tag="ps_small", name=f"ws_{l}_{hh_}")
                    nc.tensor.matmul(ws_ps[:], awt_sb[:, k * H:(k + 1) * H],
                                     ascol[:, k:k + 1], start=True, stop=True)
                    nc.vector.tensor_copy(
                        wbig[:, l * WB + H + NH + hh_:l * WB + H + NH + hh_ + 1],
                        ws_ps[:])
                    ct_ps = pss.tile([1, 1], f32, tag="ps_small", name=f"ct_{l}_{hh_}")
                    nc.tensor.matmul(ct_ps[:], wbcol[:HD, k:k + 1],
                                     adcol[:, k:k + 1], start=True, stop=True)
                    nc.vector.tensor_copy(consts[:, l * WB + H + hh_:l * WB + H + hh_ + 1],
                                          ct_ps[:])
                    cs_ps = pss.tile([1, 1], f32, tag="ps_small", name=f"cs_{l}_{hh_}")
                    nc.tensor.matmul(cs_ps[:], wbcol[:HD, k:k + 1],
                                     ascol[:, k:k + 1], start=True, stop=True)
                    # c_s + ab  (bias is per-partition [1,1])
                    nc.scalar.activation(
                        consts[:, l * WB + H + NH + hh_:l * WB + H + NH + hh_ + 1],
                        cs_ps[:], AF.Identity, bias=ab_sb[:, k:k + 1])

            nc.vector.tensor_copy(wbigb[:], wbig[:])
            nc.vector.tensor_copy(constsb[:], consts[:])

            # ---------- xT + encoder (h0 full and local) ----------
            xTf = pp.tile([F, N], f32)
            for i in range(4):
                nc.sync.dma_start(out=xTf[:, i * 1024:(i + 1) * 1024],
                                  in_=xT_d[:, i * 1024:(i + 1) * 1024])
            xT = pp.tile([F, N], bf16)
            nc.vector.tensor_copy(xT[:], xTf[:])
            xTlf = pp.tile([F, R], f32)
            nc.sync.dma_start(out=xTlf[:], in_=xTl_d)
            xTl = pp.tile([F, R], bf16)
            nc.vector.tensor_copy(xTl[:], xTlf[:])

            # ---------- adjT load + bf16 cast (cast on idle GpSimd) ----------
            adjT = pp.tile([128, NT * R], bf16)
            NCHUNK = 16
            CW = NT * R // NCHUNK  # 1024 cols per chunk
            for i in range(NCHUNK):
                ach = chp.tile([128, CW], f32, tag="ach", name=f"ach_{i}")
                nc.sync.dma_start(out=ach[:], in_=adjT_d[:, i * CW:(i + 1) * CW])
                nc.gpsimd.tensor_copy(adjT[:, i * CW:(i + 1) * CW], ach[:])

            h_full = pp.tile([128, NT * H], bf16)
            h_loc = [pp.tile([128, RT * H], f32, name=f"h_loc_{i}") for i in range(2)]
            hT_full = pp.tile([128, NT * H], bf16)
            hT_loc = pp.tile([128, R], bf16)

            for jt in range(NT):
                h0p = pss.tile([128, H], f32, tag="ps_small", name=f"h0p_{jt}")
                nc.tensor.matmul(h0p[:], xT[:, jt * 128:(jt + 1) * 128], encW[:],
                                 start=True, stop=False)
                nc.tensor.matmul(h0p[:], ones_row[:], encb[:], start=False, stop=True)
                nc.scalar.activation(h_full[:, jt * H:(jt + 1) * H], h0p[:], AF.Relu)
            for rt in range(RT):
                h0p = pss.tile([128, H], f32, tag="ps_small", name=f"h0pl_{rt}")
                nc.tensor.matmul(h0p[:], xTl[:, rt * 128:(rt + 1) * 128], encW[:],
                                 start=True, stop=False)
                nc.tensor.matmul(h0p[:], ones_row[:], encb[:], start=False, stop=True)
                nc.scalar.activation(h_loc[0][:, rt * H:(rt + 1) * H], h0p[:], AF.Relu)

            # ---------- deg (local rows) + allgather -> deg_full ----------
            deg_loc = pp.tile([128, RT], f32)
            for rt in range(RT):
                dps = pss.tile([128, 1], f32, tag="ps_small", name=f"degp_{rt}")
                for jt in range(NT):
                    nc.tensor.matmul(dps[:], adjT[:, jt * R + rt * 128:jt * R + (rt + 1) * 128],
                                     ones_colb[:], start=(jt == 0), stop=(jt == NT - 1))
                nc.vector.tensor_copy(deg_loc[:, rt:rt + 1], dps[:])
            cc_dego = dp.tile([N, 1], f32, addr_space="Shared")
            cc_degi = dp.tile([R, 1], f32)
            nc.sync.dma_start(out=cc_degi.rearrange("(t p) o -> p t o", p=128),
                              in_=deg_loc[:].unsqueeze(2))
            nc.gpsimd.collective_compute(
                "AllGather", mybir.AluOpType.bypass,
                replica_groups=[list(range(NCORES))],
                ins=[cc_degi.opt()], outs=[cc_dego.opt()])
            deg_full = pp.tile([128, NT], f32)
            nc.sync.dma_start(out=deg_full[:],
                              in_=cc_dego.rearrange("(t p) o -> p (t o)", p=128))

            # dinv = 1/sqrt(deg+1); Ndeg = N - deg
            dinv_full = pp.tile([128, NT], f32)
            nc.scalar.activation(dinv_full[:], deg_full[:], AF.Sqrt, bias=1.0)
            nc.vector.reciprocal(dinv_full[:], dinv_full[:])
            dinv_loc = pp.tile([128, RT], f32)
            nc.scalar.activation(dinv_loc[:], deg_loc[:], AF.Sqrt, bias=1.0)
            nc.vector.reciprocal(dinv_loc[:], dinv_loc[:])
            dinv2_loc = pp.tile([128, RT], f32)
            nc.vector.tensor_tensor(dinv2_loc[:], dinv_loc[:], dinv_loc[:], op=mult)
            ndeg_loc = pp.tile([128, RT], f32)
            nc.vector.tensor_scalar(ndeg_loc[:], deg_loc[:], -1.0, float(N),
                                    op0=mult, op1=add)

            # ---------- layers ----------
            for l in range(L):
                hf = h_full
                hl = h_loc[l % 2]
                hl_new = h_loc[(l + 1) % 2]
                wb_l = wbig[:, l * WB:(l + 1) * WB]
                co_l = consts[:, l * WB:(l + 1) * WB]
                wbb_l = wbigb[:, l * WB:(l + 1) * WB]
                cob_l = constsb[:, l * WB:(l + 1) * WB]

                # local transpose of h_loc for s/t (bf16 for the bf16 hh matmul)
                hlb = wp.tile([128, RT * H], bf16, tag="hlb", name=f"hlb_{l}")
                nc.vector.tensor_copy(hlb[:], hl[:])
                for rt in range(RT):
                    tp = pss.tile([128, 128], bf16, tag="ps_small", name=f"tp_{l}_{rt}")
                    nc.tensor.transpose(tp[:], hlb[:, rt * H:(rt + 1) * H], identb[:])
                    nc.vector.tensor_copy(hT_loc[:, rt * 128:(rt + 1) * 128], tp[:])

                hacc = wp.tile([128, NT], f32, tag="hacc", name=f"hacc_{l}")

                # Y accumulators (4 r-tiles x 388 cols)
                ybank = [psy.tile([128, MW], f32, tag="y", name=f"y_{l}_{rt}")
                         for rt in range(RT)]

                for jt in range(NT):
                    # hT tile (bf16 transpose: 1 cycle/row)
                    tp = pss.tile([128, 128], bf16, tag="ps_small", name=f"tph_{l}_{jt}")
                    nc.tensor.transpose(tp[:], hf[:, jt * H:(jt + 1) * H], identb[:])
                    nc.vector.tensor_scalar(hT_full[:, jt * H:(jt + 1) * H], tp[:],
                                            1.0, 0.0, op0=mult,
                                            op1=mybir.AluOpType.add,
                                            accum_out=hacc[:, jt:jt + 1])
                    # hh/t/s for this j-tile (bf16 matmul, f32 accumulate)
                    hhp = pss.tile([128, WB], f32, tag="ps_small", name=f"hhp_{l}_{jt}")
                    nc.tensor.matmul(hhp[:], hT_full[:, jt * H:(jt + 1) * H], wbb_l,
                                     start=True, stop=False)
                    nc.tensor.matmul(hhp[:], ones_row[:], cob_l, start=False, stop=True)
                    # assemble M tile (bf16)
                    mhi = mp.tile([128, MW], bf16, tag="mhi", name=f"mhi_{l}_{jt}")
                    # u = exp(t)
                    nc.scalar.activation(mhi[:, 384:388], hhp[:, H:H + NH], AF.Exp)
                    # h' = dinv * h
                    nc.vector.tensor_scalar_mul(mhi[:, 0:H], hf[:, jt * H:(jt + 1) * H],
                                                dinv_full[:, jt:jt + 1])
                    # u * hh (broadcast u over the 32 cols of each head)
                    ub = mhi[:, 384:388].unsqueeze(2).broadcast_to([128, NH, HD])
                    nc.vector.tensor_tensor(
                        mhi[:, H:2 * H].rearrange("p (h d) -> p h d", h=NH),
                        hhp[:, 0:H].rearrange("p (h d) -> p h d", h=NH), ub, op=mult)
                    # -hh
                    nc.scalar.mul(mhi[:, 2 * H:3 * H], hhp[:, 0:H], -1.0)
                    # big matmuls
                    for rt in range(RT):
                        lhs = adjT[:, jt * R + rt * 128:jt * R + (rt + 1) * 128]
                        nc.tensor.matmul(ybank[rt][:], lhs, mhi[:],
                                         start=(jt == 0), stop=False)

                # S_row388 = [0 | 0 | S_all | 0]
                hsum = wp.tile([128, 1], f32, tag="hsum", name=f"hsum_{l}")
                nc.vector.tensor_reduce(out=hsum[:], in_=hacc[:],
                                        op=add, axis=mybir.AxisListType.X)
                sraw = pss.tile([1, WB], f32, tag="ps_small", name=f"sraw_{l}")
                nc.tensor.matmul(sraw[:], hsum[:], wb_l, start=True, stop=False)
                nc.tensor.matmul(sraw[:], ncell[:], co_l, start=False, stop=True)
                srow = wp.tile([1, MW], f32, tag="srow", name=f"srow_{l}")
                nc.vector.memset(srow[:], 0.0)
                nc.vector.tensor_copy(srow[:, 2 * H:3 * H], sraw[:, 0:H])
                srhi = wp.tile([1, MW], bf16, tag="srhi", name=f"srhi_{l}")
                nc.vector.tensor_copy(srhi[:], srow[:])
                srlo = wp.tile([1, MW], bf16, tag="srlo", name=f"srlo_{l}")
                nc.vector.tensor_tensor(srlo[:], srow[:], srhi[:],
                                        op=mybir.AluOpType.subtract)
                for rt in range(RT):
                    nc.tensor.matmul(ybank[rt][:], ones_row[:], srhi[:],
                                     start=False, stop=False)
                    nc.tensor.matmul(ybank[rt][:], ones_row[:], srlo[:],
                                     start=False, stop=True)

                # epilogue per r-tile
                for rt in range(RT):
                    y = ybank[rt]
                    # s/t for local rows; es = exp(s + c_s + ab)
                    stp = pss.tile([128, 2 * NH], f32, tag="ps_small",
                                   name=f"stp_{l}_{rt}")
                    nc.tensor.matmul(stp[:], hT_loc[:, rt * 128:(rt + 1) * 128],
                                     wbb_l[:, H:], start=True, stop=False)
                    nc.tensor.matmul(stp[:], ones_row[:], cob_l[:, H:],
                                     start=False, stop=True)
                    es = wp.tile([128, 2 * NH], f32, tag="es", name=f"es_{l}_{rt}")
                    nc.scalar.activation(es[:], stp[:], AF.Exp)
                    # attention numerator: es*P + (S_all - Q)
                    sq = wp.tile([128, H], f32, tag="sq", name=f"sq_{l}_{rt}")
                    nc.scalar.copy(sq[:], y[:, 2 * H:3 * H])
                    pre = wp.tile([128, H], f32, tag="pre", name=f"pre_{l}_{rt}")
                    for hh_ in range(NH):
                        nc.vector.scalar_tensor_tensor(
                            pre[:, hh_ * HD:(hh_ + 1) * HD],
                            y[:, H + hh_ * HD:H + (hh_ + 1) * HD],
                            es[:, NH + hh_:NH + hh_ + 1],
                            sq[:, hh_ * HD:(hh_ + 1) * HD],
                            op0=mult, op1=add)
                    # Z = es*AU + (N - deg); rec = 1/Z
                    zt = wp.tile([128, NH], f32, tag="zt", name=f"zt_{l}_{rt}")
                    nc.vector.tensor_tensor(zt[:], y[:, 3 * H:3 * H + NH],
                                            es[:, NH:2 * NH], op=mult)
                    nc.vector.tensor_scalar_add(zt[:], zt[:],
                                                ndeg_loc[:, rt:rt + 1])
                    nc.vector.reciprocal(zt[:], zt[:])
                    # GCN: sup = dinv*(A1 + h'_loc) = dinv*A1 + dinv^2*h_loc
                    hp2 = wp.tile([128, H], f32, tag="hp2", name=f"hp2_{l}_{rt}")
                    nc.vector.tensor_scalar_mul(hp2[:], hl[:, rt * H:(rt + 1) * H],
                                                dinv2_loc[:, rt:rt + 1])
                    sup = wp.tile([128, H], f32, tag="sup", name=f"sup_{l}_{rt}")
                    nc.vector.scalar_tensor_tensor(sup[:], y[:, 0:H],
                                                   dinv_loc[:, rt:rt + 1], hp2[:],
                                                   op0=mult, op1=add)
                    supt = pss.tile([128, 128], f32, tag="ps_small",
                                    name=f"supt_{l}_{rt}")
                    nc.tensor.transpose(supt[:], sup[:], ident[:])
                    supts = wp.tile([128, 128], f32, tag="supts", name=f"supts_{l}_{rt}")
                    nc.scalar.copy(supts[:], supt[:])
                    gcnp = pss.tile([128, H], f32, tag="ps_small", name=f"gcnp_{l}_{rt}")
                    nc.tensor.matmul(gcnp[:], supts[:], gcnW[:, l * H:(l + 1) * H],
                                     start=True, stop=False)
                    nc.tensor.matmul(gcnp[:], ones_row[:], gcnb[:, l * H:(l + 1) * H],
                                     start=False, stop=True)
                    hg = wp.tile([128, H], f32, tag="hg", name=f"hg_{l}_{rt}")
                    nc.scalar.activation(hg[:], gcnp[:], AF.Relu)
                    # h_new = relu(hg + pre * rec)
                    for hh_ in range(NH):
                        nc.vector.scalar_tensor_tensor(
                            pre[:, hh_ * HD:(hh_ + 1) * HD],
                            pre[:, hh_ * HD:(hh_ + 1) * HD],
                            zt[:, hh_:hh_ + 1],
                            hg[:, hh_ * HD:(hh_ + 1) * HD], op0=mult, op1=add)
                    nc.scalar.activation(hl_new[:, rt * H:(rt + 1) * H], pre[:],
                                         AF.Relu)
                    # staggered allgather of this r-block (bf16)
                    if l < L - 1:
                        hlnb = wp.tile([128, H], bf16, tag="hlnb",
                                       name=f"hlnb_{l}_{rt}")
                        nc.vector.tensor_copy(hlnb[:],
                                              hl_new[:, rt * H:(rt + 1) * H])
                        cci = dp.tile([128, H], bf16, name=f"cci_{l}_{rt}")
                        cco = dp.tile([8 * 128, H], bf16, addr_space="Shared",
                                      name=f"cco_{l}_{rt}")
                        nc.sync.dma_start(out=cci[:], in_=hlnb[:])
                        nc.gpsimd.collective_compute(
                            "AllGather", mybir.AluOpType.bypass,
                            replica_groups=[list(range(NCORES))],
                            ins=[cci.opt()], outs=[cco.opt()])
                        # core c's block lands in j-tile 4c + rt
                        ccot = cco.rearrange("(c p) f -> p c f", p=128)
                        for c_ in range(8):
                            jt_ = 4 * c_ + rt
                            nc.sync.dma_start(
                                out=h_full[:, jt_ * H:(jt_ + 1) * H],
                                in_=ccot[:, c_, :])

                if l == L - 1:
                    # contagion: local column-sum -> tiny allgather -> mean MLP
                    meanp = pss.tile([128, 1], f32, tag="ps_small", name="meanp")
                    for rt in range(RT):
                        nc.tensor.matmul(meanp[:], hl_new[:, rt * H:(rt + 1) * H],
                                         ones_colf[:], start=(rt == 0),
                                         stop=(rt == RT - 1))
                    mloc = wp.tile([128, 1], f32, tag="mloc", name="mloc")
                    nc.vector.tensor_copy(mloc[:], meanp[:])
                    mgi = dp.tile([128, 1], f32, name="mgi")
                    mgo = dp.tile([8 * 128, 1], f32, addr_space="Shared", name="mgo")
                    nc.sync.dma_start(out=mgi[:], in_=mloc[:])
                    nc.gpsimd.collective_compute(
                        "AllGather", mybir.AluOpType.bypass,
                        replica_groups=[list(range(NCORES))],
                        ins=[mgi.opt()], outs=[mgo.opt()])
                    mall = wp.tile([128, 8], f32, tag="mall", name="mall")
                    nc.sync.dma_start(out=mall[:],
                                      in_=mgo.rearrange("(c p) u -> p (c u)", p=128))
                    msum = wp.tile([128, 1], f32, tag="msum", name="msum")
                    nc.vector.tensor_reduce(out=msum[:], in_=mall[:],
                                            op=add, axis=mybir.AxisListType.X)
                    means = wp.tile([128, 1], f32, tag="means", name="means")
                    nc.scalar.mul(means[:], msum[:], 1.0 / N)
                    c1p = pss.tile([H // 2, 1], f32, tag="ps_small", name="c1p")
                    nc.tensor.matmul(c1p[:], nW1[:], means[:], start=True, stop=True)
                    c1s = wp.tile([H // 2, 1], f32, tag="c1s", name="c1s")
                    nc.scalar.activation(c1s[:], c1p[:], AF.Relu, bias=nb1[:])
                    c2p = pss.tile([1, 1], f32, tag="ps_small", name="c2p")
                    nc.tensor.matmul(c2p[:], nW2[:], c1s[:], start=True, stop=True)
                    c2s = wp.tile([1, 1], f32, tag="c2s", name="c2s")
                    nc.scalar.activation(c2s[:], c2p[:], AF.Identity, bias=nb2[:])
                    nc.sync.dma_start(out=outc_d, in_=c2s[:])

            # ---------- node outputs (from final h_loc = h_loc[L % 2]) ----------
            hfin = h_loc[L % 2]
            nc.sync.dma_start(out=outh_d.rearrange("(t p) c -> p t c", p=128),
                              in_=hfin[:].rearrange("p (t c) -> p t c", t=RT))
            hTfin = pp.tile([128, R], f32)
            for rt in range(RT):
                tp = pss.tile([128, 128], f32, tag="ps_small", name=f"tpf_{rt}")
                nc.tensor.transpose(tp[:], hfin[:, rt * H:(rt + 1) * H], ident[:])
                nc.scalar.copy(hTfin[:, rt * 128:(rt + 1) * 128], tp[:])
            z1p = pss.tile([H // 2, R], f32, tag="ps_small", name="z1p")
            nc.tensor.matmul(z1p[:], cW1[:], hTfin[:], start=True, stop=True)
            z1s = wp.tile([H // 2, R], f32, tag="z1s", name="z1s")
            nc.scalar.activation(z1s[:], z1p[:], AF.Relu, bias=cb1[:])
            lgp = pss.tile([CDIM, R], f32, tag="ps_small", name="lgp")
            nc.tensor.matmul(lgp[:], cW2[:], z1s[:], start=True, stop=True)
            lgs = wp.tile([CDIM, R], f32, tag="lgs", name="lgs")
            nc.scalar.activation(lgs[:], lgp[:], AF.Identity, bias=cb2[:])
            nc.sync.dma_start(out=logT_d, in_=lgs[:])

    nc.compile()
    return nc


def _install_ntff_hook():
    """The image's antenv lacks axon_hooks; inject it so trace=True works."""
    import sys
    import types

    try:
        from antenv.axon_hooks import get_axon_ntff_profile_hook  # noqa: F401
        return
    except ImportError:
        pass
    import antenv
    mod = types.ModuleType("antenv.axon_hooks")
    state = {"hook": None}
    mod.set_axon_ntff_profile_hook = lambda h: state.__setitem__("hook", h)
    mod.get_axon_ntff_profile_hook = lambda: state["hook"]
    sys.modules["antenv.axon_hooks"] = mod
    antenv.axon_hooks = mod
    try:
        from trn_agent_boot.trn_boot import _ntff_profile_via_ctypes
        mod.set_axon_ntff_profile_hook(
            _ntff_profile_via_ctypes("/opt/axon/libaxon_pjrt.so"))
    except Exception:
        pass


def _host_prep(inputs):
    """Shard/layout the full inputs per core (pure layout, no arithmetic)."""
    I = {k: np.ascontiguousarray(np.asarray(v, dtype=np.float32))
         for k, v in inputs.items()}
    adj = I["adj"]
    x = I["x"]
    xT = np.ascontiguousarray(x.T)
    adjTf = np.ascontiguousarray(adj.T)  # [j, i]
    attn_W = I["attn_W"]
    shared = {
        "xT": xT,
        "enc_W": I["enc_W"], "enc_b": I["enc_b"],
        "gcn_W": I["gcn_W"], "gcn_b": I["gcn_b"],
        "attn_Wcat": np.ascontiguousarray(
            attn_W.transpose(0, 2, 1, 3).reshape(L, H, H)),
        "attn_WT": np.ascontiguousarray(attn_W.transpose(0, 1, 3, 2)),
        "attn_Wb": I["attn_Wb"], "attn_a": I["attn_a"], "attn_ab": I["attn_ab"],
        "cls_W1": I["cls_W1"], "cls_b1": I["cls_b1"],
        "cls_W2": I["cls_W2"], "cls_b2": I["cls_b2"],
        "con_W1": I["con_W1"], "con_b1": I["con_b1"],
        "con_W2": I["con_W2"], "con_b2": I["con_b2"],
    }
    in_maps = []
    for c in range(NCORES):
        blk = adjTf[:, c * R:(c + 1) * R]  # [4096, 512]
        adjT_dev = np.ascontiguousarray(
            blk.reshape(NT, 128, R).transpose(1, 0, 2).reshape(128, NT * R))
        m = dict(shared)
        m["adjT"] = adjT_dev
        m["xT_loc"] = np.ascontiguousarray(xT[:, c * R:(c + 1) * R])
        in_maps.append(m)
    return in_maps


def run(inputs, trace=False):
    from concourse import bass_utils
    _install_ntff_hook()
    if "nc" not in _CACHE:
        _CACHE["nc"] = _build_nc()
    nc = _CACHE["nc"]
    in_maps = _host_prep(inputs)
    res = bass_utils.run_bass_kernel_spmd(
        nc, in_maps, core_ids=list(range(NCORES)), trace=trace)
    node_logits = np.concatenate(
        [np.ascontiguousarray(res.results[c]["logitsT"].T) for c in range(NCORES)],
        axis=0)
    h = np.concatenate([res.results[c]["out_h"] for c in range(NCORES)], axis=0)
    contagion = res.results[0]["out_con"]
    return (node_logits, h, contagion), res


def kernel(**inputs):
    (node_logits, h, contagion), _ = run(inputs, trace=False)
    return (node_logits.astype(np.float32), h.astype(np.float32),
            contagion.astype(np.float32))


# revision 8
# speedup vs baseline: 1.1351x; 1.1351x over previous
"""GNN message-passing kernel for 8 TRN2 NeuronCores.

Reference: 3 layers of (GCN + 4-head graph attention) over a 4096x4096
adjacency, plus encoder / classifier / contagion heads.

Algebraic reformulation of the attention (validated to ~1e-6 vs the
reference): non-edges get score exactly 0 (exp=1), so with u = exp(t),
es_i = exp(s_i + ab):

    probs @ hh = [es * (adj @ (u*hh)) + (S_all - adj @ hh)] / Z
    Z          = es * (adj @ u) + (N - deg)

Everything reduces to row-blocks of adj @ M where M is [N, 388]:
    cols [0:128)    dinv * h          (GCN, symmetric-normalized)
    cols [128:256)  u * hh            (4 heads x 32)
    cols [256:384)  -hh               (negated; ones x S_all added on top)
    cols [384:388)  u                 (4 heads)

Sharding: core c owns rows [512c, 512c+512). Its stationary operand is
adj[rows_c, :].T laid out k-major in SBUF as bf16 (adj is 0/1 -> exact).
M is split hi/lo into two bf16 matmuls accumulated in the same PSUM bank
(~1.5e-5 relative precision). h (N x 128) is all-gathered each layer.
"""

import numpy as np

NCORES = 8
N = 4096
F = 64
H = 128
NH = 4
HD = 32
L = 3
CDIM = 7
R = N // NCORES          # 512 rows per core
NT = N // 128            # 32 j-tiles
RT = R // 128            # 4 r-tiles per core
MW = H + H + H + NH      # 388 columns of M
USE_SPLIT = False        # bf16-only M measured at 4.6e-4 rel err (gate 2e-2)

_CACHE = {}


def _build_nc():
    import concourse.bass as bass
    import concourse.mybir as mybir
    from concourse import bacc
    import concourse.tile as tile
    from concourse.masks import make_identity

    dt = mybir.dt
    f32 = dt.float32
    bf16 = dt.bfloat16
    AF = mybir.ActivationFunctionType

    nc = bacc.Bacc("TRN2", target_bir_lowering=False, debug=False,
                   num_devices=NCORES)

    # ---- I/O ----
    adjT_d = nc.dram_tensor("adjT", [128, NT * R], f32, kind="ExternalInput").ap()
    xT_d = nc.dram_tensor("xT", [F, N], f32, kind="ExternalInput").ap()
    xTl_d = nc.dram_tensor("xT_loc", [F, R], f32, kind="ExternalInput").ap()
    encW_d = nc.dram_tensor("enc_W", [F, H], f32, kind="ExternalInput").ap()
    encb_d = nc.dram_tensor("enc_b", [H], f32, kind="ExternalInput").ap()
    gcnW_d = nc.dram_tensor("gcn_W", [L, H, H], f32, kind="ExternalInput").ap()
    gcnb_d = nc.dram_tensor("gcn_b", [L, H], f32, kind="ExternalInput").ap()
    awcat_d = nc.dram_tensor("attn_Wcat", [L, H, H], f32, kind="ExternalInput").ap()
    awt_d = nc.dram_tensor("attn_WT", [L, NH, HD, H], f32, kind="ExternalInput").ap()
    awb_d = nc.dram_tensor("attn_Wb", [L, NH, HD], f32, kind="ExternalInput").ap()
    aa_d = nc.dram_tensor("attn_a", [L, NH, 2 * HD], f32, kind="ExternalInput").ap()
    aab_d = nc.dram_tensor("attn_ab", [L, NH], f32, kind="ExternalInput").ap()
    cW1_d = nc.dram_tensor("cls_W1", [H, H // 2], f32, kind="ExternalInput").ap()
    cb1_d = nc.dram_tensor("cls_b1", [H // 2], f32, kind="ExternalInput").ap()
    cW2_d = nc.dram_tensor("cls_W2", [H // 2, CDIM], f32, kind="ExternalInput").ap()
    cb2_d = nc.dram_tensor("cls_b2", [CDIM], f32, kind="ExternalInput").ap()
    nW1_d = nc.dram_tensor("con_W1", [H, H // 2], f32, kind="ExternalInput").ap()
    nb1_d = nc.dram_tensor("con_b1", [H // 2], f32, kind="ExternalInput").ap()
    nW2_d = nc.dram_tensor("con_W2", [H // 2, 1], f32, kind="ExternalInput").ap()
    nb2_d = nc.dram_tensor("con_b2", [1], f32, kind="ExternalInput").ap()

    logT_d = nc.dram_tensor("logitsT", [CDIM, R], f32, kind="ExternalOutput").ap()
    outh_d = nc.dram_tensor("out_h", [R, H], f32, kind="ExternalOutput").ap()
    outc_d = nc.dram_tensor("out_con", [1, 1], f32, kind="ExternalOutput").ap()

    mult = mybir.AluOpType.mult
    add = mybir.AluOpType.add

    with tile.TileContext(nc) as tc:
        with (
            tc.tile_pool(name="persist", bufs=1) as pp,
            tc.tile_pool(name="chunk", bufs=2) as chp,
            tc.tile_pool(name="mpool", bufs=3) as mp,
            tc.tile_pool(name="work", bufs=3) as wp,
            tc.tile_pool(name="ps_y", bufs=4, space="PSUM") as psy,
            tc.tile_pool(name="ps_s", bufs=4, space="PSUM") as pss,
            tc.tile_pool(name="dram", bufs=1, space="DRAM") as dp,
        ):
            # ---------- constants ----------
            ident = pp.tile([128, 128], f32)
            make_identity(nc, ident)
            identb = pp.tile([128, 128], bf16)
            nc.vector.tensor_copy(identb[:], ident[:])
            ones_row = pp.tile([1, 128], bf16)
            nc.vector.memset(ones_row[:], 1.0)
            ones_rowf = pp.tile([1, 128], f32)
            nc.vector.memset(ones_rowf[:], 1.0)
            ncell = pp.tile([1, 1], f32)
            nc.vector.memset(ncell[:], float(N))
            ones_colb = pp.tile([128, 1], bf16)
            nc.vector.memset(ones_colb[:], 1.0)
            ones_colf = pp.tile([128, 1], f32)
            nc.vector.memset(ones_colf[:], 1.0)

            # ---------- small weights ----------
            encWf = pp.tile([F, H], f32)
            nc.sync.dma_start(out=encWf[:], in_=encW_d)
            encW = pp.tile([F, H], bf16)
            nc.vector.tensor_copy(encW[:], encWf[:])
            encbf = pp.tile([1, H], f32)
            nc.sync.dma_start(out=encbf[:], in_=encb_d.unsqueeze(0))
            encb = pp.tile([1, H], bf16)
            nc.vector.tensor_copy(encb[:], encbf[:])
            gcnW = pp.tile([128, L * H], f32)
            gcnbf = pp.tile([1, L * H], f32)
            for l in range(L):
                nc.sync.dma_start(out=gcnW[:, l * H:(l + 1) * H], in_=gcnW_d[l])
                nc.sync.dma_start(out=gcnbf[:, l * H:(l + 1) * H],
                                  in_=gcnb_d[l].unsqueeze(0))
            gcnb = pp.tile([1, L * H], bf16)
            nc.vector.tensor_copy(gcnb[:], gcnbf[:])
            cW1 = pp.tile([H, H // 2], f32)
            nc.sync.dma_start(out=cW1[:], in_=cW1_d)
            cb1 = pp.tile([H // 2, 1], f32)
            nc.sync.dma_start(out=cb1[:], in_=cb1_d.unsqueeze(1))
            cW2 = pp.tile([H // 2, CDIM], f32)
            nc.sync.dma_start(out=cW2[:], in_=cW2_d)
            cb2 = pp.tile([CDIM, 1], f32)
            nc.sync.dma_start(out=cb2[:], in_=cb2_d.unsqueeze(1))
            nW1 = pp.tile([H, H // 2], f32)
            nc.sync.dma_start(out=nW1[:], in_=nW1_d)
            nb1 = pp.tile([H // 2, 1], f32)
            nc.sync.dma_start(out=nb1[:], in_=nb1_d.unsqueeze(1))
            nW2 = pp.tile([H // 2, 1], f32)
            nc.sync.dma_start(out=nW2[:], in_=nW2_d)
            nb2 = pp.tile([1, 1], f32)
            nc.sync.dma_start(out=nb2[:], in_=nb2_d.unsqueeze(0))
            ab_sb = pp.tile([1, L * NH], f32)
            nc.sync.dma_start(out=ab_sb[:], in_=aab_d.flatten().unsqueeze(0))

            # per-(l,h) columns used to fold s/t into the hh matmul
            awt_sb = pp.tile([HD, L * NH * H], f32)
            adcol = pp.tile([HD, L * NH], f32)   # a_dst columns
            ascol = pp.tile([HD, L * NH], f32)   # a_src columns
            wbcol = pp.tile([HD, L * NH], f32)   # attn_Wb columns
            for l in range(L):
                for hh_ in range(NH):
                    k = l * NH + hh_
                    nc.sync.dma_start(out=awt_sb[:, k * H:(k + 1) * H],
                                      in_=awt_d[l, hh_])
                    nc.sync.dma_start(out=adcol[:, k:k + 1],
                                      in_=aa_d[l, hh_, HD:].unsqueeze(1))
                    nc.sync.dma_start(out=ascol[:, k:k + 1],
                                      in_=aa_d[l, hh_, 0:HD].unsqueeze(1))
                    nc.sync.dma_start(out=wbcol[:, k:k + 1],
                                      in_=awb_d[l, hh_].unsqueeze(1))

            # W_big[l] = [Wcat (128) | w_t (4) | w_s (4)]  and consts rows
            WB = H + 2 * NH  # 136
            wbig = pp.tile([128, L * WB], f32)
            wbigb = pp.tile([128, L * WB], bf16)
            consts = pp.tile([1, L * WB], f32)
            constsb = pp.tile([1, L * WB], bf16)
            for l in range(L):
                nc.sync.dma_start(out=wbig[:, l * WB:l * WB + H], in_=awcat_d[l])
                nc.sync.dma_start(out=consts[:, l * WB:l * WB + H],
                                  in_=awb_d[l].flatten().unsqueeze(0))
                for hh_ in range(NH):
                    k = l * NH + hh_
                    wt_ps = pss.tile([128, 1], f32, tag="ps_small", name=f"wt_{l}_{hh_}")
                    nc.tensor.matmul(wt_ps[:], awt_sb[:, k * H:(k + 1) * H],
                                     adcol[:, k:k + 1], start=True, stop=True)
                    nc.vector.tensor_copy(wbig[:, l * WB + H + hh_:l * WB + H + hh_ + 1],
                                          wt_ps[:])
                    ws_ps = pss.tile([128, 1], f32, tag="ps_small", name=f"ws_{l}_{hh_}")
                    nc.tensor.matmul(ws_ps[:], awt_sb[:, k * H:(k + 1) * H],
                                     ascol[:, k:k + 1], start=True, stop=True)
                    nc.vector.tensor_copy(
                        wbig[:, l * WB + H + NH + hh_:l * WB + H + NH + hh_ + 1],
                        ws_ps[:])
                    ct_ps = pss.tile([1, 1], f32, tag="ps_small", name=f"ct_{l}_{hh_}")
                    nc.tensor.matmul(ct_ps[:], wbcol[:HD, k:k + 1],
                                     adcol[:, k:k + 1], start=True, stop=True)
                    nc.vector.tensor_copy(consts[:, l * WB + H + hh_:l * WB + H + hh_ + 1],
                                          ct_ps[:])
                    cs_ps = pss.tile([1, 1], f32, tag="ps_small", name=f"cs_{l}_{hh_}")
                    nc.tensor.matmul(cs_ps[:], wbcol[:HD, k:k + 1],
                                     ascol[:, k:k + 1], start=True, stop=True)
                    # c_s + ab  (bias is per-partition [1,1])
                    nc.scalar.activation(
                        consts[:, l * WB + H + NH + hh_:l * WB + H + NH + hh_ + 1],
                        cs_ps[:], AF.Identity, bias=ab_sb[:, k:k + 1])

            nc.vector.tensor_copy(wbigb[:], wbig[:])
            nc.vector.tensor_copy(constsb[:], consts[:])

            # ---------- xT + encoder (h0 full and local) ----------
            xTf = pp.tile([F, N], f32)
            for i in range(4):
                nc.sync.dma_start(out=xTf[:, i * 1024:(i + 1) * 1024],
                                  in_=xT_d[:, i * 1024:(i + 1) * 1024])
            xT = pp.tile([F, N], bf16)
            nc.vector.tensor_copy(xT[:], xTf[:])
            xTlf = pp.tile([F, R], f32)
            nc.sync.dma_start(out=xTlf[:], in_=xTl_d)
            xTl = pp.tile([F, R], bf16)
            nc.vector.tensor_copy(xTl[:], xTlf[:])

            # ---------- adjT load + bf16 cast (cast on idle GpSimd) ----------
            adjT = pp.tile([128, NT * R], bf16)
            NCHUNK = 16
            CW = NT * R // NCHUNK  # 1024 cols per chunk
            for i in range(NCHUNK):
                ach = chp.tile([128, CW], f32, tag="ach", name=f"ach_{i}")
                nc.sync.dma_start(out=ach[:], in_=adjT_d[:, i * CW:(i + 1) * CW])
                if i % 2 == 0:
                    nc.scalar.copy(adjT[:, i * CW:(i + 1) * CW], ach[:])
                else:
                    nc.vector.tensor_copy(adjT[:, i * CW:(i + 1) * CW], ach[:])

            h_full = pp.tile([128, NT * H], bf16)
            h_loc = [pp.tile([128, RT * H], f32, name=f"h_loc_{i}") for i in range(2)]
            hT_full = pp.tile([128, NT * H], bf16)
            hT_loc = pp.tile([128, R], bf16)

            for jt in range(NT):
                h0p = pss.tile([128, H], f32, tag="ps_small", name=f"h0p_{jt}")
                nc.tensor.matmul(h0p[:], xT[:, jt * 128:(jt + 1) * 128], encW[:],
                                 start=True, stop=False)
                nc.tensor.matmul(h0p[:], ones_row[:], encb[:], start=False, stop=True)
                nc.scalar.activation(h_full[:, jt * H:(jt + 1) * H], h0p[:], AF.Relu)
            for rt in range(RT):
                h0p = pss.tile([128, H], f32, tag="ps_small", name=f"h0pl_{rt}")
                nc.tensor.matmul(h0p[:], xTl[:, rt * 128:(rt + 1) * 128], encW[:],
                                 start=True, stop=False)
                nc.tensor.matmul(h0p[:], ones_row[:], encb[:], start=False, stop=True)
                nc.scalar.activation(h_loc[0][:, rt * H:(rt + 1) * H], h0p[:], AF.Relu)

            # ---------- deg (local rows) + allgather -> deg_full ----------
            deg_loc = pp.tile([128, RT], f32)
            for rt in range(RT):
                dps = pss.tile([128, 1], f32, tag="ps_small", name=f"degp_{rt}")
                for jt in range(NT):
                    nc.tensor.matmul(dps[:], adjT[:, jt * R + rt * 128:jt * R + (rt + 1) * 128],
                                     ones_colb[:], start=(jt == 0), stop=(jt == NT - 1))
                nc.vector.tensor_copy(deg_loc[:, rt:rt + 1], dps[:])
            cc_dego = dp.tile([N, 1], f32, addr_space="Shared")
            cc_degi = dp.tile([R, 1], f32)
            nc.sync.dma_start(out=cc_degi.rearrange("(t p) o -> p t o", p=128),
                              in_=deg_loc[:].unsqueeze(2))
            nc.gpsimd.collective_compute(
                "AllGather", mybir.AluOpType.bypass,
                replica_groups=[list(range(NCORES))],
                ins=[cc_degi.opt()], outs=[cc_dego.opt()])
            deg_full = pp.tile([128, NT], f32)
            nc.sync.dma_start(out=deg_full[:],
                              in_=cc_dego.rearrange("(t p) o -> p (t o)", p=128))

            # dinv = 1/sqrt(deg+1); Ndeg = N - deg
            dinv_full = pp.tile([128, NT], f32)
            nc.scalar.activation(dinv_full[:], deg_full[:], AF.Sqrt, bias=1.0)
            nc.vector.reciprocal(dinv_full[:], dinv_full[:])
            dinv_loc = pp.tile([128, RT], f32)
            nc.scalar.activation(dinv_loc[:], deg_loc[:], AF.Sqrt, bias=1.0)
            nc.vector.reciprocal(dinv_loc[:], dinv_loc[:])
            dinv2_loc = pp.tile([128, RT], f32)
            nc.vector.tensor_tensor(dinv2_loc[:], dinv_loc[:], dinv_loc[:], op=mult)
            ndeg_loc = pp.tile([128, RT], f32)
            nc.vector.tensor_scalar(ndeg_loc[:], deg_loc[:], -1.0, float(N),
                                    op0=mult, op1=add)

            # ---------- layers ----------
            for l in range(L):
                hf = h_full
                hl = h_loc[l % 2]
                hl_new = h_loc[(l + 1) % 2]
                wb_l = wbig[:, l * WB:(l + 1) * WB]
                co_l = consts[:, l * WB:(l + 1) * WB]
                wbb_l = wbigb[:, l * WB:(l + 1) * WB]
                cob_l = constsb[:, l * WB:(l + 1) * WB]

                # local transpose of h_loc for s/t (bf16 for the bf16 hh matmul)
                hlb = wp.tile([128, RT * H], bf16, tag="hlb", name=f"hlb_{l}")
                nc.vector.tensor_copy(hlb[:], hl[:])
                for rt in range(RT):
                    tp = pss.tile([128, 128], bf16, tag="ps_small", name=f"tp_{l}_{rt}")
                    nc.tensor.transpose(tp[:], hlb[:, rt * H:(rt + 1) * H], identb[:])
                    nc.vector.tensor_copy(hT_loc[:, rt * 128:(rt + 1) * 128], tp[:])

                hacc = wp.tile([128, NT], f32, tag="hacc", name=f"hacc_{l}")

                # Y accumulators (4 r-tiles x 388 cols)
                ybank = [psy.tile([128, MW], f32, tag="y", name=f"y_{l}_{rt}")
                         for rt in range(RT)]

                for jt in range(NT):
                    # hT tile (bf16 transpose: 1 cycle/row)
                    tp = pss.tile([128, 128], bf16, tag="ps_small", name=f"tph_{l}_{jt}")
                    nc.tensor.transpose(tp[:], hf[:, jt * H:(jt + 1) * H], identb[:])
                    nc.vector.tensor_scalar(hT_full[:, jt * H:(jt + 1) * H], tp[:],
                                            1.0, 0.0, op0=mult,
                                            op1=mybir.AluOpType.add,
                                            accum_out=hacc[:, jt:jt + 1])
                    # hh/t/s for this j-tile (bf16 matmul, f32 accumulate)
                    hhp = pss.tile([128, WB], f32, tag="ps_small", name=f"hhp_{l}_{jt}")
                    nc.tensor.matmul(hhp[:], hT_full[:, jt * H:(jt + 1) * H], wbb_l,
                                     start=True, stop=False)
                    nc.tensor.matmul(hhp[:], ones_row[:], cob_l, start=False, stop=True)
                    # assemble M tile (bf16)
                    mhi = mp.tile([128, MW], bf16, tag="mhi", name=f"mhi_{l}_{jt}")
                    # u = exp(t)
                    nc.scalar.activation(mhi[:, 384:388], hhp[:, H:H + NH], AF.Exp)
                    # h' = dinv * h
                    nc.vector.tensor_scalar_mul(mhi[:, 0:H], hf[:, jt * H:(jt + 1) * H],
                                                dinv_full[:, jt:jt + 1])
                    # u * hh (broadcast u over the 32 cols of each head)
                    ub = mhi[:, 384:388].unsqueeze(2).broadcast_to([128, NH, HD])
                    nc.vector.tensor_tensor(
                        mhi[:, H:2 * H].rearrange("p (h d) -> p h d", h=NH),
                        hhp[:, 0:H].rearrange("p (h d) -> p h d", h=NH), ub, op=mult)
                    # -hh
                    nc.scalar.mul(mhi[:, 2 * H:3 * H], hhp[:, 0:H], -1.0)
                    # big matmuls; layer 0 splits off the dinv-dependent
                    # h' columns so the rest can start before the deg gather
                    for rt in range(RT):
                        lhs = adjT[:, jt * R + rt * 128:jt * R + (rt + 1) * 128]
                        if l == 0:
                            nc.tensor.matmul(ybank[rt][:, H:], lhs, mhi[:, H:],
                                             start=(jt == 0), stop=False)
                            nc.tensor.matmul(ybank[rt][:, 0:H], lhs, mhi[:, 0:H],
                                             start=(jt == 0), stop=False)
                        else:
                            nc.tensor.matmul(ybank[rt][:], lhs, mhi[:],
                                             start=(jt == 0), stop=False)

                # S_row388 = [0 | 0 | S_all | 0]
                hsum = wp.tile([128, 1], f32, tag="hsum", name=f"hsum_{l}")
                nc.vector.tensor_reduce(out=hsum[:], in_=hacc[:],
                                        op=add, axis=mybir.AxisListType.X)
                sraw = pss.tile([1, WB], f32, tag="ps_small", name=f"sraw_{l}")
                nc.tensor.matmul(sraw[:], hsum[:], wb_l, start=True, stop=False)
                nc.tensor.matmul(sraw[:], ncell[:], co_l, start=False, stop=True)
                srow = wp.tile([1, MW], f32, tag="srow", name=f"srow_{l}")
                nc.vector.memset(srow[:], 0.0)
                nc.vector.tensor_copy(srow[:, 2 * H:3 * H], sraw[:, 0:H])
                srhi = wp.tile([1, MW], bf16, tag="srhi", name=f"srhi_{l}")
                nc.vector.tensor_copy(srhi[:], srow[:])
                srlo = wp.tile([1, MW], bf16, tag="srlo", name=f"srlo_{l}")
                nc.vector.tensor_tensor(srlo[:], srow[:], srhi[:],
                                        op=mybir.AluOpType.subtract)
                for rt in range(RT):
                    nc.tensor.matmul(ybank[rt][:], ones_row[:], srhi[:],
                                     start=False, stop=False)
                    nc.tensor.matmul(ybank[rt][:], ones_row[:], srlo[:],
                                     start=False, stop=True)

                # epilogue per r-tile
                for rt in range(RT):
                    y = ybank[rt]
                    # s/t for local rows; es = exp(s + c_s + ab)
                    stp = pss.tile([128, 2 * NH], f32, tag="ps_small",
                                   name=f"stp_{l}_{rt}")
                    nc.tensor.matmul(stp[:], hT_loc[:, rt * 128:(rt + 1) * 128],
                                     wbb_l[:, H:], start=True, stop=False)
                    nc.tensor.matmul(stp[:], ones_row[:], cob_l[:, H:],
                                     start=False, stop=True)
                    es = wp.tile([128, 2 * NH], f32, tag="es", name=f"es_{l}_{rt}")
                    nc.scalar.activation(es[:], stp[:], AF.Exp)
                    # attention numerator: es*P + (S_all - Q)
                    sq = wp.tile([128, H], f32, tag="sq", name=f"sq_{l}_{rt}")
                    nc.scalar.copy(sq[:], y[:, 2 * H:3 * H])
                    pre = wp.tile([128, H], f32, tag="pre", name=f"pre_{l}_{rt}")
                    for hh_ in range(NH):
                        nc.vector.scalar_tensor_tensor(
                            pre[:, hh_ * HD:(hh_ + 1) * HD],
                            y[:, H + hh_ * HD:H + (hh_ + 1) * HD],
                            es[:, NH + hh_:NH + hh_ + 1],
                            sq[:, hh_ * HD:(hh_ + 1) * HD],
                            op0=mult, op1=add)
                    # Z = es*AU + (N - deg); rec = 1/Z
                    zt = wp.tile([128, NH], f32, tag="zt", name=f"zt_{l}_{rt}")
                    nc.vector.tensor_tensor(zt[:], y[:, 3 * H:3 * H + NH],
                                            es[:, NH:2 * NH], op=mult)
                    nc.vector.tensor_scalar_add(zt[:], zt[:],
                                                ndeg_loc[:, rt:rt + 1])
                    nc.vector.reciprocal(zt[:], zt[:])
                    # GCN: sup = dinv*(A1 + h'_loc) = dinv*A1 + dinv^2*h_loc
                    hp2 = wp.tile([128, H], f32, tag="hp2", name=f"hp2_{l}_{rt}")
                    nc.vector.tensor_scalar_mul(hp2[:], hl[:, rt * H:(rt + 1) * H],
                                                dinv2_loc[:, rt:rt + 1])
                    sup = wp.tile([128, H], f32, tag="sup", name=f"sup_{l}_{rt}")
                    nc.vector.scalar_tensor_tensor(sup[:], y[:, 0:H],
                                                   dinv_loc[:, rt:rt + 1], hp2[:],
                                                   op0=mult, op1=add)
                    supt = pss.tile([128, 128], f32, tag="ps_small",
                                    name=f"supt_{l}_{rt}")
                    nc.tensor.transpose(supt[:], sup[:], ident[:])
                    supts = wp.tile([128, 128], f32, tag="supts", name=f"supts_{l}_{rt}")
                    nc.scalar.copy(supts[:], supt[:])
                    gcnp = pss.tile([128, H], f32, tag="ps_small", name=f"gcnp_{l}_{rt}")
                    nc.tensor.matmul(gcnp[:], supts[:], gcnW[:, l * H:(l + 1) * H],
                                     start=True, stop=False)
                    nc.tensor.matmul(gcnp[:], ones_row[:], gcnb[:, l * H:(l + 1) * H],
                                     start=False, stop=True)
                    hg = wp.tile([128, H], f32, tag="hg", name=f"hg_{l}_{rt}")
                    nc.scalar.activation(hg[:], gcnp[:], AF.Relu)
                    # h_new = relu(hg + pre * rec)
                    for hh_ in range(NH):
                        nc.vector.scalar_tensor_tensor(
                            pre[:, hh_ * HD:(hh_ + 1) * HD],
                            pre[:, hh_ * HD:(hh_ + 1) * HD],
                            zt[:, hh_:hh_ + 1],
                            hg[:, hh_ * HD:(hh_ + 1) * HD], op0=mult, op1=add)
                    nc.scalar.activation(hl_new[:, rt * H:(rt + 1) * H], pre[:],
                                         AF.Relu)

                if l < L - 1:
                    # allgather h in bf16 (halves collective + reload traffic)
                    hlnb = wp.tile([128, RT * H], bf16, tag="hlnb", name=f"hlnb_{l}")
                    nc.vector.tensor_copy(hlnb[:], hl_new[:])
                    cci = dp.tile([R, H], bf16, name=f"cci_{l}")
                    cco = dp.tile([N, H], bf16, addr_space="Shared", name=f"cco_{l}")
                    nc.sync.dma_start(out=cci.rearrange("(t p) c -> p t c", p=128),
                                      in_=hlnb[:].rearrange("p (t c) -> p t c", t=RT))
                    nc.gpsimd.collective_compute(
                        "AllGather", mybir.AluOpType.bypass,
                        replica_groups=[list(range(NCORES))],
                        ins=[cci.opt()], outs=[cco.opt()])
                    ccot = cco.rearrange("(t p) c -> p t c", p=128)
                    for i in range(8):
                        nc.sync.dma_start(
                            out=h_full[:, i * 4 * H:(i + 1) * 4 * H]
                                .rearrange("p (t c) -> p t c", t=4),
                            in_=ccot[:, i * 4:(i + 1) * 4, :])

                if l == L - 1:
                    # contagion: local column-sum -> tiny allgather -> mean MLP
                    meanp = pss.tile([128, 1], f32, tag="ps_small", name="meanp")
                    for rt in range(RT):
                        nc.tensor.matmul(meanp[:], hl_new[:, rt * H:(rt + 1) * H],
                                         ones_colf[:], start=(rt == 0),
                                         stop=(rt == RT - 1))
                    mloc = wp.tile([128, 1], f32, tag="mloc", name="mloc")
                    nc.vector.tensor_copy(mloc[:], meanp[:])
                    mgi = dp.tile([128, 1], f32, name="mgi")
                    mgo = dp.tile([8 * 128, 1], f32, addr_space="Shared", name="mgo")
                    nc.sync.dma_start(out=mgi[:], in_=mloc[:])
                    nc.gpsimd.collective_compute(
                        "AllGather", mybir.AluOpType.bypass,
                        replica_groups=[list(range(NCORES))],
                        ins=[mgi.opt()], outs=[mgo.opt()])
                    mall = wp.tile([128, 8], f32, tag="mall", name="mall")
                    nc.sync.dma_start(out=mall[:],
                                      in_=mgo.rearrange("(c p) u -> p (c u)", p=128))
                    msum = wp.tile([128, 1], f32, tag="msum", name="msum")
                    nc.vector.tensor_reduce(out=msum[:], in_=mall[:],
                                            op=add, axis=mybir.AxisListType.X)
                    means = wp.tile([128, 1], f32, tag="means", name="means")
                    nc.scalar.mul(means[:], msum[:], 1.0 / N)
                    c1p = pss.tile([H // 2, 1], f32, tag="ps_small", name="c1p")
                    nc.tensor.matmul(c1p[:], nW1[:], means[:], start=True, stop=True)
                    c1s = wp.tile([H // 2, 1], f32, tag="c1s", name="c1s")
                    nc.scalar.activation(c1s[:], c1p[:], AF.Relu, bias=nb1[:])
                    c2p = pss.tile([1, 1], f32, tag="ps_small", name="c2p")
                    nc.tensor.matmul(c2p[:], nW2[:], c1s[:], start=True, stop=True)
                    c2s = wp.tile([1, 1], f32, tag="c2s", name="c2s")
                    nc.scalar.activation(c2s[:], c2p[:], AF.Identity, bias=nb2[:])
                    nc.sync.dma_start(out=outc_d, in_=c2s[:])

            # ---------- node outputs (from final h_loc = h_loc[L % 2]) ----------
            hfin = h_loc[L % 2]
            nc.sync.dma_start(out=outh_d.rearrange("(t p) c -> p t c", p=128),
                              in_=hfin[:].rearrange("p (t c) -> p t c", t=RT))
            hTfin = pp.tile([128, R], f32)
            for rt in range(RT):
                tp = pss.tile([128, 128], f32, tag="ps_small", name=f"tpf_{rt}")
                nc.tensor.transpose(tp[:], hfin[:, rt * H:(rt + 1) * H], ident[:])
                nc.scalar.copy(hTfin[:, rt * 128:(rt + 1) * 128], tp[:])
            z1p = pss.tile([H // 2, R], f32, tag="ps_small", name="z1p")
            nc.tensor.matmul(z1p[:], cW1[:], hTfin[:], start=True, stop=True)
            z1s = wp.tile([H // 2, R], f32, tag="z1s", name="z1s")
            nc.scalar.activation(z1s[:], z1p[:], AF.Relu, bias=cb1[:])
            lgp = pss.tile([CDIM, R], f32, tag="ps_small", name="lgp")
            nc.tensor.matmul(lgp[:], cW2[:], z1s[:], start=True, stop=True)
            lgs = wp.tile([CDIM, R], f32, tag="lgs", name="lgs")
            nc.scalar.activation(lgs[:], lgp[:], AF.Identity, bias=cb2[:])
            nc.sync.dma_start(out=logT_d, in_=lgs[:])

    nc.compile()
    return nc


def _install_ntff_hook():
    """The image's antenv lacks axon_hooks; inject it so trace=True works."""
    import sys
    import types

    try:
        from antenv.axon_hooks import get_axon_ntff_profile_hook  # noqa: F401
        return
    except ImportError:
        pass
    import antenv
    mod = types.ModuleType("antenv.axon_hooks")
    state = {"hook": None}
    mod.set_axon_ntff_profile_hook = lambda h: state.__setitem__("hook", h)
    mod.get_axon_ntff_profile_hook = lambda: state["hook"]
    sys.modules["antenv.axon_hooks"] = mod
    antenv.axon_hooks = mod
    try:
        from trn_agent_boot.trn_boot import _ntff_profile_via_ctypes
        mod.set_axon_ntff_profile_hook(
            _ntff_profile_via_ctypes("/opt/axon/libaxon_pjrt.so"))
    except Exception:
        pass


def _host_prep(inputs):
    """Shard/layout the full inputs per core (pure layout, no arithmetic)."""
    I = {k: np.ascontiguousarray(np.asarray(v, dtype=np.float32))
         for k, v in inputs.items()}
    adj = I["adj"]
    x = I["x"]
    xT = np.ascontiguousarray(x.T)
    adjTf = np.ascontiguousarray(adj.T)  # [j, i]
    attn_W = I["attn_W"]
    shared = {
        "xT": xT,
        "enc_W": I["enc_W"], "enc_b": I["enc_b"],
        "gcn_W": I["gcn_W"], "gcn_b": I["gcn_b"],
        "attn_Wcat": np.ascontiguousarray(
            attn_W.transpose(0, 2, 1, 3).reshape(L, H, H)),
        "attn_WT": np.ascontiguousarray(attn_W.transpose(0, 1, 3, 2)),
        "attn_Wb": I["attn_Wb"], "attn_a": I["attn_a"], "attn_ab": I["attn_ab"],
        "cls_W1": I["cls_W1"], "cls_b1": I["cls_b1"],
        "cls_W2": I["cls_W2"], "cls_b2": I["cls_b2"],
        "con_W1": I["con_W1"], "con_b1": I["con_b1"],
        "con_W2": I["con_W2"], "con_b2": I["con_b2"],
    }
    in_maps = []
    for c in range(NCORES):
        blk = adjTf[:, c * R:(c + 1) * R]  # [4096, 512]
        adjT_dev = np.ascontiguousarray(
            blk.reshape(NT, 128, R).transpose(1, 0, 2).reshape(128, NT * R))
        m = dict(shared)
        m["adjT"] = adjT_dev
        m["xT_loc"] = np.ascontiguousarray(xT[:, c * R:(c + 1) * R])
        in_maps.append(m)
    return in_maps


def run(inputs, trace=False):
    from concourse import bass_utils
    _install_ntff_hook()
    if "nc" not in _CACHE:
        _CACHE["nc"] = _build_nc()
    nc = _CACHE["nc"]
    in_maps = _host_prep(inputs)
    res = bass_utils.run_bass_kernel_spmd(
        nc, in_maps, core_ids=list(range(NCORES)), trace=trace)
    node_logits = np.concatenate(
        [np.ascontiguousarray(res.results[c]["logitsT"].T) for c in range(NCORES)],
        axis=0)
    h = np.concatenate([res.results[c]["out_h"] for c in range(NCORES)], axis=0)
    contagion = res.results[0]["out_con"]
    return (node_logits, h, contagion), res


def kernel(**inputs):
    (node_logits, h, contagion), _ = run(inputs, trace=False)
    return (node_logits.astype(np.float32), h.astype(np.float32),
            contagion.astype(np.float32))


# revision 9
# speedup vs baseline: 1.1468x; 1.0103x over previous
"""GNN message-passing kernel for 8 TRN2 NeuronCores.

Reference: 3 layers of (GCN + 4-head graph attention) over a 4096x4096
adjacency, plus encoder / classifier / contagion heads.

Algebraic reformulation of the attention (validated to ~1e-6 vs the
reference): non-edges get score exactly 0 (exp=1), so with u = exp(t),
es_i = exp(s_i + ab):

    probs @ hh = [es * (adj @ (u*hh)) + (S_all - adj @ hh)] / Z
    Z          = es * (adj @ u) + (N - deg)

Everything reduces to row-blocks of adj @ M where M is [N, 388]:
    cols [0:128)    dinv * h          (GCN, symmetric-normalized)
    cols [128:256)  u * hh            (4 heads x 32)
    cols [256:384)  -hh               (negated; ones x S_all added on top)
    cols [384:388)  u                 (4 heads)

Sharding: core c owns rows [512c, 512c+512). Its stationary operand is
adj[rows_c, :].T laid out k-major in SBUF as bf16 (adj is 0/1 -> exact).
M is split hi/lo into two bf16 matmuls accumulated in the same PSUM bank
(~1.5e-5 relative precision). h (N x 128) is all-gathered each layer.
"""

import numpy as np

NCORES = 8
N = 4096
F = 64
H = 128
NH = 4
HD = 32
L = 3
CDIM = 7
R = N // NCORES          # 512 rows per core
NT = N // 128            # 32 j-tiles
RT = R // 128            # 4 r-tiles per core
MW = H + H + H + NH      # 388 columns of M
USE_SPLIT = False        # bf16-only M measured at 4.6e-4 rel err (gate 2e-2)

_CACHE = {}


def _build_nc():
    import concourse.bass as bass
    import concourse.mybir as mybir
    from concourse import bacc
    import concourse.tile as tile
    from concourse.masks import make_identity

    dt = mybir.dt
    f32 = dt.float32
    bf16 = dt.bfloat16
    AF = mybir.ActivationFunctionType

    nc = bacc.Bacc("TRN2", target_bir_lowering=False, debug=False,
                   num_devices=NCORES)

    # ---- I/O ----
    adjT_d = nc.dram_tensor("adjT", [128, NT * R], f32, kind="ExternalInput").ap()
    xT_d = nc.dram_tensor("xT", [F, N], f32, kind="ExternalInput").ap()
    xTl_d = nc.dram_tensor("xT_loc", [F, R], f32, kind="ExternalInput").ap()
    encW_d = nc.dram_tensor("enc_W", [F, H], f32, kind="ExternalInput").ap()
    encb_d = nc.dram_tensor("enc_b", [H], f32, kind="ExternalInput").ap()
    gcnW_d = nc.dram_tensor("gcn_W", [L, H, H], f32, kind="ExternalInput").ap()
    gcnb_d = nc.dram_tensor("gcn_b", [L, H], f32, kind="ExternalInput").ap()
    awcat_d = nc.dram_tensor("attn_Wcat", [L, H, H], f32, kind="ExternalInput").ap()
    awt_d = nc.dram_tensor("attn_WT", [L, NH, HD, H], f32, kind="ExternalInput").ap()
    awb_d = nc.dram_tensor("attn_Wb", [L, NH, HD], f32, kind="ExternalInput").ap()
    aa_d = nc.dram_tensor("attn_a", [L, NH, 2 * HD], f32, kind="ExternalInput").ap()
    aab_d = nc.dram_tensor("attn_ab", [L, NH], f32, kind="ExternalInput").ap()
    cW1_d = nc.dram_tensor("cls_W1", [H, H // 2], f32, kind="ExternalInput").ap()
    cb1_d = nc.dram_tensor("cls_b1", [H // 2], f32, kind="ExternalInput").ap()
    cW2_d = nc.dram_tensor("cls_W2", [H // 2, CDIM], f32, kind="ExternalInput").ap()
    cb2_d = nc.dram_tensor("cls_b2", [CDIM], f32, kind="ExternalInput").ap()
    nW1_d = nc.dram_tensor("con_W1", [H, H // 2], f32, kind="ExternalInput").ap()
    nb1_d = nc.dram_tensor("con_b1", [H // 2], f32, kind="ExternalInput").ap()
    nW2_d = nc.dram_tensor("con_W2", [H // 2, 1], f32, kind="ExternalInput").ap()
    nb2_d = nc.dram_tensor("con_b2", [1], f32, kind="ExternalInput").ap()

    logT_d = nc.dram_tensor("logitsT", [CDIM, R], f32, kind="ExternalOutput").ap()
    outh_d = nc.dram_tensor("out_h", [R, H], f32, kind="ExternalOutput").ap()
    outc_d = nc.dram_tensor("out_con", [1, 1], f32, kind="ExternalOutput").ap()

    mult = mybir.AluOpType.mult
    add = mybir.AluOpType.add

    with tile.TileContext(nc) as tc:
        with (
            tc.tile_pool(name="persist", bufs=1) as pp,
            tc.tile_pool(name="chunk", bufs=2) as chp,
            tc.tile_pool(name="mpool", bufs=34) as mp,
            tc.tile_pool(name="work", bufs=3) as wp,
            tc.tile_pool(name="ps_y", bufs=4, space="PSUM") as psy,
            tc.tile_pool(name="ps_s", bufs=4, space="PSUM") as pss,
            tc.tile_pool(name="dram", bufs=1, space="DRAM") as dp,
        ):
            # ---------- constants ----------
            ident = pp.tile([128, 128], f32)
            make_identity(nc, ident)
            identb = pp.tile([128, 128], bf16)
            nc.vector.tensor_copy(identb[:], ident[:])
            ones_row = pp.tile([1, 128], bf16)
            nc.vector.memset(ones_row[:], 1.0)
            ones_rowf = pp.tile([1, 128], f32)
            nc.vector.memset(ones_rowf[:], 1.0)
            ncell = pp.tile([1, 1], f32)
            nc.vector.memset(ncell[:], float(N))
            ones_colb = pp.tile([128, 1], bf16)
            nc.vector.memset(ones_colb[:], 1.0)
            ones_colf = pp.tile([128, 1], f32)
            nc.vector.memset(ones_colf[:], 1.0)

            # ---------- small weights ----------
            encWf = pp.tile([F, H], f32)
            nc.sync.dma_start(out=encWf[:], in_=encW_d)
            encW = pp.tile([F, H], bf16)
            nc.vector.tensor_copy(encW[:], encWf[:])
            encbf = pp.tile([1, H], f32)
            nc.sync.dma_start(out=encbf[:], in_=encb_d.unsqueeze(0))
            encb = pp.tile([1, H], bf16)
            nc.vector.tensor_copy(encb[:], encbf[:])
            xTf = pp.tile([F, N], f32)
            for i in range(4):
                nc.sync.dma_start(out=xTf[:, i * 1024:(i + 1) * 1024],
                                  in_=xT_d[:, i * 1024:(i + 1) * 1024])
            xT = pp.tile([F, N], bf16)
            nc.vector.tensor_copy(xT[:], xTf[:])
            xTlf = pp.tile([F, R], f32)
            nc.sync.dma_start(out=xTlf[:], in_=xTl_d)
            xTl = pp.tile([F, R], bf16)
            nc.vector.tensor_copy(xTl[:], xTlf[:])

            gcnW = pp.tile([128, L * H], f32)
            gcnbf = pp.tile([1, L * H], f32)
            for l in range(L):
                nc.sync.dma_start(out=gcnW[:, l * H:(l + 1) * H], in_=gcnW_d[l])
                nc.sync.dma_start(out=gcnbf[:, l * H:(l + 1) * H],
                                  in_=gcnb_d[l].unsqueeze(0))
            gcnb = pp.tile([1, L * H], bf16)
            nc.vector.tensor_copy(gcnb[:], gcnbf[:])
            cW1 = pp.tile([H, H // 2], f32)
            nc.sync.dma_start(out=cW1[:], in_=cW1_d)
            cb1 = pp.tile([H // 2, 1], f32)
            nc.sync.dma_start(out=cb1[:], in_=cb1_d.unsqueeze(1))
            cW2 = pp.tile([H // 2, CDIM], f32)
            nc.sync.dma_start(out=cW2[:], in_=cW2_d)
            cb2 = pp.tile([CDIM, 1], f32)
            nc.sync.dma_start(out=cb2[:], in_=cb2_d.unsqueeze(1))
            nW1 = pp.tile([H, H // 2], f32)
            nc.sync.dma_start(out=nW1[:], in_=nW1_d)
            nb1 = pp.tile([H // 2, 1], f32)
            nc.sync.dma_start(out=nb1[:], in_=nb1_d.unsqueeze(1))
            nW2 = pp.tile([H // 2, 1], f32)
            nc.sync.dma_start(out=nW2[:], in_=nW2_d)
            nb2 = pp.tile([1, 1], f32)
            nc.sync.dma_start(out=nb2[:], in_=nb2_d.unsqueeze(0))
            ab_sb = pp.tile([1, L * NH], f32)
            nc.sync.dma_start(out=ab_sb[:], in_=aab_d.flatten().unsqueeze(0))

            # per-(l,h) columns used to fold s/t into the hh matmul
            awt_sb = pp.tile([HD, L * NH * H], f32)
            adcol = pp.tile([HD, L * NH], f32)   # a_dst columns
            ascol = pp.tile([HD, L * NH], f32)   # a_src columns
            wbcol = pp.tile([HD, L * NH], f32)   # attn_Wb columns
            for l in range(L):
                for hh_ in range(NH):
                    k = l * NH + hh_
                    nc.sync.dma_start(out=awt_sb[:, k * H:(k + 1) * H],
                                      in_=awt_d[l, hh_])
                    nc.sync.dma_start(out=adcol[:, k:k + 1],
                                      in_=aa_d[l, hh_, HD:].unsqueeze(1))
                    nc.sync.dma_start(out=ascol[:, k:k + 1],
                                      in_=aa_d[l, hh_, 0:HD].unsqueeze(1))
                    nc.sync.dma_start(out=wbcol[:, k:k + 1],
                                      in_=awb_d[l, hh_].unsqueeze(1))

            # W_big[l] = [Wcat (128) | w_t (4) | w_s (4)]  and consts rows
            WB = H + 2 * NH  # 136
            wbig = pp.tile([128, L * WB], f32)
            wbigb = pp.tile([128, L * WB], bf16)
            consts = pp.tile([1, L * WB], f32)
            constsb = pp.tile([1, L * WB], bf16)
            for l in range(L):
                nc.sync.dma_start(out=wbig[:, l * WB:l * WB + H], in_=awcat_d[l])
                nc.sync.dma_start(out=consts[:, l * WB:l * WB + H],
                                  in_=awb_d[l].flatten().unsqueeze(0))
                for hh_ in range(NH):
                    k = l * NH + hh_
                    wt_ps = pss.tile([128, 1], f32, tag="ps_small", name=f"wt_{l}_{hh_}")
                    nc.tensor.matmul(wt_ps[:], awt_sb[:, k * H:(k + 1) * H],
                                     adcol[:, k:k + 1], start=True, stop=True)
                    nc.vector.tensor_copy(wbig[:, l * WB + H + hh_:l * WB + H + hh_ + 1],
                                          wt_ps[:])
                    ws_ps = pss.tile([128, 1], f32, tag="ps_small", name=f"ws_{l}_{hh_}")
                    nc.tensor.matmul(ws_ps[:], awt_sb[:, k * H:(k + 1) * H],
                                     ascol[:, k:k + 1], start=True, stop=True)
                    nc.vector.tensor_copy(
                        wbig[:, l * WB + H + NH + hh_:l * WB + H + NH + hh_ + 1],
                        ws_ps[:])
                    ct_ps = pss.tile([1, 1], f32, tag="ps_small", name=f"ct_{l}_{hh_}")
                    nc.tensor.matmul(ct_ps[:], wbcol[:HD, k:k + 1],
                                     adcol[:, k:k + 1], start=True, stop=True)
                    nc.vector.tensor_copy(consts[:, l * WB + H + hh_:l * WB + H + hh_ + 1],
                                          ct_ps[:])
                    cs_ps = pss.tile([1, 1], f32, tag="ps_small", name=f"cs_{l}_{hh_}")
                    nc.tensor.matmul(cs_ps[:], wbcol[:HD, k:k + 1],
                                     ascol[:, k:k + 1], start=True, stop=True)
                    # c_s + ab  (bias is per-partition [1,1])
                    nc.scalar.activation(
                        consts[:, l * WB + H + NH + hh_:l * WB + H + NH + hh_ + 1],
                        cs_ps[:], AF.Identity, bias=ab_sb[:, k:k + 1])

            nc.vector.tensor_copy(wbigb[:], wbig[:])
            nc.vector.tensor_copy(constsb[:], consts[:])

            # ---------- xT + encoder (h0 full and local) ----------
            # ---------- adjT load + bf16 cast (cast on idle GpSimd) ----------
            adjT = pp.tile([128, NT * R], bf16)
            NCHUNK = 16
            CW = NT * R // NCHUNK  # 1024 cols per chunk
            for i in range(NCHUNK):
                ach = chp.tile([128, CW], f32, tag="ach", name=f"ach_{i}")
                nc.sync.dma_start(out=ach[:], in_=adjT_d[:, i * CW:(i + 1) * CW])
                if i % 2 == 0:
                    nc.scalar.copy(adjT[:, i * CW:(i + 1) * CW], ach[:])
                else:
                    nc.vector.tensor_copy(adjT[:, i * CW:(i + 1) * CW], ach[:])

            h_full = pp.tile([128, NT * H], bf16)
            h_loc = [pp.tile([128, RT * H], f32, name=f"h_loc_{i}") for i in range(2)]
            hT_full = pp.tile([128, NT * H], bf16)
            hT_loc = pp.tile([128, R], bf16)

            h0_dram = dp.tile([N, H], bf16, name="h0_dram")
            h0dt = h0_dram.rearrange("(t p) c -> p t c", p=128)
            for jt in range(NT):
                h0p = pss.tile([128, H], f32, tag="ps_small", name=f"h0p_{jt}")
                nc.tensor.matmul(h0p[:], xT[:, jt * 128:(jt + 1) * 128], encW[:],
                                 start=True, stop=False)
                nc.tensor.matmul(h0p[:], ones_row[:], encb[:], start=False, stop=True)
                nc.scalar.activation(h_full[:, jt * H:(jt + 1) * H], h0p[:], AF.Relu)
                nc.sync.dma_start(out=h0dt[:, jt, :],
                                  in_=h_full[:, jt * H:(jt + 1) * H])
            for rt in range(RT):
                h0p = pss.tile([128, H], f32, tag="ps_small", name=f"h0pl_{rt}")
                nc.tensor.matmul(h0p[:], xTl[:, rt * 128:(rt + 1) * 128], encW[:],
                                 start=True, stop=False)
                nc.tensor.matmul(h0p[:], ones_row[:], encb[:], start=False, stop=True)
                nc.scalar.activation(h_loc[0][:, rt * H:(rt + 1) * H], h0p[:], AF.Relu)

            # ---------- deg (local rows) + allgather -> deg_full ----------
            deg_loc = pp.tile([128, RT], f32)
            for rt in range(RT):
                dps = pss.tile([128, 1], f32, tag="ps_small", name=f"degp_{rt}")
                for jt in range(NT):
                    nc.tensor.matmul(dps[:], adjT[:, jt * R + rt * 128:jt * R + (rt + 1) * 128],
                                     ones_colb[:], start=(jt == 0), stop=(jt == NT - 1))
                nc.vector.tensor_copy(deg_loc[:, rt:rt + 1], dps[:])
            cc_dego = dp.tile([N, 1], f32, addr_space="Shared")
            cc_degi = dp.tile([R, 1], f32)
            nc.sync.dma_start(out=cc_degi.rearrange("(t p) o -> p t o", p=128),
                              in_=deg_loc[:].unsqueeze(2))
            nc.gpsimd.collective_compute(
                "AllGather", mybir.AluOpType.bypass,
                replica_groups=[list(range(NCORES))],
                ins=[cc_degi.opt()], outs=[cc_dego.opt()])
            deg_full = pp.tile([128, NT], f32)
            nc.sync.dma_start(out=deg_full[:],
                              in_=cc_dego.rearrange("(t p) o -> p (t o)", p=128))

            # dinv = 1/sqrt(deg+1); Ndeg = N - deg
            dinv_full = pp.tile([128, NT], f32)
            nc.scalar.activation(dinv_full[:], deg_full[:], AF.Sqrt, bias=1.0)
            nc.vector.reciprocal(dinv_full[:], dinv_full[:])
            dinv_loc = pp.tile([128, RT], f32)
            nc.scalar.activation(dinv_loc[:], deg_loc[:], AF.Sqrt, bias=1.0)
            nc.vector.reciprocal(dinv_loc[:], dinv_loc[:])
            dinv2_loc = pp.tile([128, RT], f32)
            nc.vector.tensor_tensor(dinv2_loc[:], dinv_loc[:], dinv_loc[:], op=mult)
            ndeg_loc = pp.tile([128, RT], f32)
            nc.vector.tensor_scalar(ndeg_loc[:], deg_loc[:], -1.0, float(N),
                                    op0=mult, op1=add)

            # ---------- layers ----------
            hsrc_dram = [h0_dram]  # per-layer DRAM copy of h (bf16), for hT
            for l in range(L):
                hf = h_full
                hl = h_loc[l % 2]
                hl_new = h_loc[(l + 1) % 2]
                wb_l = wbig[:, l * WB:(l + 1) * WB]
                co_l = consts[:, l * WB:(l + 1) * WB]
                wbb_l = wbigb[:, l * WB:(l + 1) * WB]
                cob_l = constsb[:, l * WB:(l + 1) * WB]

                # local transpose of h_loc for s/t (bf16 for the bf16 hh matmul)
                hlb = wp.tile([128, RT * H], bf16, tag="hlb", name=f"hlb_{l}")
                nc.vector.tensor_copy(hlb[:], hl[:])
                for rt in range(RT):
                    tp = pss.tile([128, 128], bf16, tag="ps_small", name=f"tp_{l}_{rt}")
                    nc.tensor.transpose(tp[:], hlb[:, rt * H:(rt + 1) * H], identb[:])
                    nc.vector.tensor_copy(hT_loc[:, rt * 128:(rt + 1) * 128], tp[:])

                # hT for this layer: hardware transpose-load from DRAM
                nc.sync.dma_start_transpose(hT_full[:], hsrc_dram[l][:])

                # Y accumulators (4 r-tiles x 388 cols)
                ybank = [psy.tile([128, MW], f32, tag="y", name=f"y_{l}_{rt}")
                         for rt in range(RT)]

                mtiles = []
                for jt in range(NT):
                    # hh/t/s for this j-tile (bf16 matmul, f32 accumulate)
                    hhp = pss.tile([128, WB], f32, tag="ps_small", name=f"hhp_{l}_{jt}")
                    nc.tensor.matmul(hhp[:], hT_full[:, jt * H:(jt + 1) * H], wbb_l,
                                     start=True, stop=False)
                    nc.tensor.matmul(hhp[:], ones_row[:], cob_l, start=False, stop=True)
                    # assemble M tile (bf16)
                    mhi = mp.tile([128, MW], bf16, tag="mhi", name=f"mhi_{l}_{jt}")
                    mtiles.append(mhi)
                    # u = exp(t)
                    nc.scalar.activation(mhi[:, 384:388], hhp[:, H:H + NH], AF.Exp)
                    # u * hh (broadcast u over the 32 cols of each head)
                    ub = mhi[:, 384:388].unsqueeze(2).broadcast_to([128, NH, HD])
                    nc.vector.tensor_tensor(
                        mhi[:, H:2 * H].rearrange("p (h d) -> p h d", h=NH),
                        hhp[:, 0:H].rearrange("p (h d) -> p h d", h=NH), ub, op=mult)
                    # -hh
                    nc.vector.tensor_scalar_mul(mhi[:, 2 * H:3 * H], hhp[:, 0:H], -1.0)
                    if l == 0:
                        # dinv (deg allgather) may not be ready yet: do the
                        # independent columns now, h' columns in a second pass
                        for rt in range(RT):
                            lhs = adjT[:, jt * R + rt * 128:jt * R + (rt + 1) * 128]
                            nc.tensor.matmul(ybank[rt][:, H:], lhs, mhi[:, H:],
                                             start=(jt == 0), stop=False)
                    else:
                        # h' = dinv * h
                        nc.vector.tensor_scalar_mul(
                            mhi[:, 0:H], hf[:, jt * H:(jt + 1) * H],
                            dinv_full[:, jt:jt + 1])
                        for rt in range(RT):
                            lhs = adjT[:, jt * R + rt * 128:jt * R + (rt + 1) * 128]
                            nc.tensor.matmul(ybank[rt][:], lhs, mhi[:],
                                             start=(jt == 0), stop=False)
                if l == 0:
                    for jt in range(NT):
                        mhi = mtiles[jt]
                        nc.vector.tensor_scalar_mul(
                            mhi[:, 0:H], hf[:, jt * H:(jt + 1) * H],
                            dinv_full[:, jt:jt + 1])
                        for rt in range(RT):
                            lhs = adjT[:, jt * R + rt * 128:jt * R + (rt + 1) * 128]
                            nc.tensor.matmul(ybank[rt][:, 0:H], lhs, mhi[:, 0:H],
                                             start=(jt == 0), stop=False)

                # S_row388 = [0 | 0 | S_all | 0]
                hsum = wp.tile([128, 1], f32, tag="hsum", name=f"hsum_{l}")
                nc.vector.tensor_reduce(out=hsum[:], in_=hT_full[:],
                                        op=add, axis=mybir.AxisListType.X)
                sraw = pss.tile([1, WB], f32, tag="ps_small", name=f"sraw_{l}")
                nc.tensor.matmul(sraw[:], hsum[:], wb_l, start=True, stop=False)
                nc.tensor.matmul(sraw[:], ncell[:], co_l, start=False, stop=True)
                srow = wp.tile([1, MW], f32, tag="srow", name=f"srow_{l}")
                nc.vector.memset(srow[:], 0.0)
                nc.vector.tensor_copy(srow[:, 2 * H:3 * H], sraw[:, 0:H])
                srhi = wp.tile([1, MW], bf16, tag="srhi", name=f"srhi_{l}")
                nc.vector.tensor_copy(srhi[:], srow[:])
                srlo = wp.tile([1, MW], bf16, tag="srlo", name=f"srlo_{l}")
                nc.vector.tensor_tensor(srlo[:], srow[:], srhi[:],
                                        op=mybir.AluOpType.subtract)
                for rt in range(RT):
                    nc.tensor.matmul(ybank[rt][:], ones_row[:], srhi[:],
                                     start=False, stop=False)
                    nc.tensor.matmul(ybank[rt][:], ones_row[:], srlo[:],
                                     start=False, stop=True)

                # epilogue per r-tile
                for rt in range(RT):
                    y = ybank[rt]
                    # s/t for local rows; es = exp(s + c_s + ab)
                    stp = pss.tile([128, 2 * NH], f32, tag="ps_small",
                                   name=f"stp_{l}_{rt}")
                    nc.tensor.matmul(stp[:], hT_loc[:, rt * 128:(rt + 1) * 128],
                                     wbb_l[:, H:], start=True, stop=False)
                    nc.tensor.matmul(stp[:], ones_row[:], cob_l[:, H:],
                                     start=False, stop=True)
                    es = wp.tile([128, 2 * NH], f32, tag="es", name=f"es_{l}_{rt}")
                    nc.scalar.activation(es[:], stp[:], AF.Exp)
                    # attention numerator: es*P + (S_all - Q)
                    sq = wp.tile([128, H], f32, tag="sq", name=f"sq_{l}_{rt}")
                    nc.scalar.copy(sq[:], y[:, 2 * H:3 * H])
                    pre = wp.tile([128, H], f32, tag="pre", name=f"pre_{l}_{rt}")
                    for hh_ in range(NH):
                        nc.vector.scalar_tensor_tensor(
                            pre[:, hh_ * HD:(hh_ + 1) * HD],
                            y[:, H + hh_ * HD:H + (hh_ + 1) * HD],
                            es[:, NH + hh_:NH + hh_ + 1],
                            sq[:, hh_ * HD:(hh_ + 1) * HD],
                            op0=mult, op1=add)
                    # Z = es*AU + (N - deg); rec = 1/Z
                    zt = wp.tile([128, NH], f32, tag="zt", name=f"zt_{l}_{rt}")
                    nc.vector.tensor_tensor(zt[:], y[:, 3 * H:3 * H + NH],
                                            es[:, NH:2 * NH], op=mult)
                    nc.vector.tensor_scalar_add(zt[:], zt[:],
                                                ndeg_loc[:, rt:rt + 1])
                    nc.vector.reciprocal(zt[:], zt[:])
                    # GCN: sup = dinv*(A1 + h'_loc) = dinv*A1 + dinv^2*h_loc
                    hp2 = wp.tile([128, H], f32, tag="hp2", name=f"hp2_{l}_{rt}")
                    nc.vector.tensor_scalar_mul(hp2[:], hl[:, rt * H:(rt + 1) * H],
                                                dinv2_loc[:, rt:rt + 1])
                    sup = wp.tile([128, H], f32, tag="sup", name=f"sup_{l}_{rt}")
                    nc.vector.scalar_tensor_tensor(sup[:], y[:, 0:H],
                                                   dinv_loc[:, rt:rt + 1], hp2[:],
                                                   op0=mult, op1=add)
                    supt = pss.tile([128, 128], f32, tag="ps_small",
                                    name=f"supt_{l}_{rt}")
                    nc.tensor.transpose(supt[:], sup[:], ident[:])
                    supts = wp.tile([128, 128], f32, tag="supts", name=f"supts_{l}_{rt}")
                    nc.scalar.copy(supts[:], supt[:])
                    gcnp = pss.tile([128, H], f32, tag="ps_small", name=f"gcnp_{l}_{rt}")
                    nc.tensor.matmul(gcnp[:], supts[:], gcnW[:, l * H:(l + 1) * H],
                                     start=True, stop=False)
                    nc.tensor.matmul(gcnp[:], ones_row[:], gcnb[:, l * H:(l + 1) * H],
                                     start=False, stop=True)
                    hg = wp.tile([128, H], f32, tag="hg", name=f"hg_{l}_{rt}")
                    nc.scalar.activation(hg[:], gcnp[:], AF.Relu)
                    # h_new = relu(hg + pre * rec)
                    for hh_ in range(NH):
                        nc.vector.scalar_tensor_tensor(
                            pre[:, hh_ * HD:(hh_ + 1) * HD],
                            pre[:, hh_ * HD:(hh_ + 1) * HD],
                            zt[:, hh_:hh_ + 1],
                            hg[:, hh_ * HD:(hh_ + 1) * HD], op0=mult, op1=add)
                    nc.scalar.activation(hl_new[:, rt * H:(rt + 1) * H], pre[:],
                                         AF.Relu)

                if l < L - 1:
                    # allgather h in bf16 (halves collective + reload traffic)
                    hlnb = wp.tile([128, RT * H], bf16, tag="hlnb", name=f"hlnb_{l}")
                    nc.vector.tensor_copy(hlnb[:], hl_new[:])
                    cci = dp.tile([R, H], bf16, name=f"cci_{l}")
                    cco = dp.tile([N, H], bf16, addr_space="Shared", name=f"cco_{l}")
                    nc.sync.dma_start(out=cci.rearrange("(t p) c -> p t c", p=128),
                                      in_=hlnb[:].rearrange("p (t c) -> p t c", t=RT))
                    nc.gpsimd.collective_compute(
                        "AllGather", mybir.AluOpType.bypass,
                        replica_groups=[list(range(NCORES))],
                        ins=[cci.opt()], outs=[cco.opt()])
                    ccot = cco.rearrange("(t p) c -> p t c", p=128)
                    for i in range(8):
                        nc.sync.dma_start(
                            out=h_full[:, i * 4 * H:(i + 1) * 4 * H]
                                .rearrange("p (t c) -> p t c", t=4),
                            in_=ccot[:, i * 4:(i + 1) * 4, :])
                    hsrc_dram.append(cco)

                if l == L - 1:
                    # contagion: local column-sum -> tiny allgather -> mean MLP
                    meanp = pss.tile([128, 1], f32, tag="ps_small", name="meanp")
                    for rt in range(RT):
                        nc.tensor.matmul(meanp[:], hl_new[:, rt * H:(rt + 1) * H],
                                         ones_colf[:], start=(rt == 0),
                                         stop=(rt == RT - 1))
                    mloc = wp.tile([128, 1], f32, tag="mloc", name="mloc")
                    nc.vector.tensor_copy(mloc[:], meanp[:])
                    mgi = dp.tile([128, 1], f32, name="mgi")
                    mgo = dp.tile([8 * 128, 1], f32, addr_space="Shared", name="mgo")
                    nc.sync.dma_start(out=mgi[:], in_=mloc[:])
                    nc.gpsimd.collective_compute(
                        "AllGather", mybir.AluOpType.bypass,
                        replica_groups=[list(range(NCORES))],
                        ins=[mgi.opt()], outs=[mgo.opt()])
                    mall = wp.tile([128, 8], f32, tag="mall", name="mall")
                    nc.sync.dma_start(out=mall[:],
                                      in_=mgo.rearrange("(c p) u -> p (c u)", p=128))
                    msum = wp.tile([128, 1], f32, tag="msum", name="msum")
                    nc.vector.tensor_reduce(out=msum[:], in_=mall[:],
                                            op=add, axis=mybir.AxisListType.X)
                    means = wp.tile([128, 1], f32, tag="means", name="means")
                    nc.scalar.mul(means[:], msum[:], 1.0 / N)
                    c1p = pss.tile([H // 2, 1], f32, tag="ps_small", name="c1p")
                    nc.tensor.matmul(c1p[:], nW1[:], means[:], start=True, stop=True)
                    c1s = wp.tile([H // 2, 1], f32, tag="c1s", name="c1s")
                    nc.scalar.activation(c1s[:], c1p[:], AF.Relu, bias=nb1[:])
                    c2p = pss.tile([1, 1], f32, tag="ps_small", name="c2p")
                    nc.tensor.matmul(c2p[:], nW2[:], c1s[:], start=True, stop=True)
                    c2s = wp.tile([1, 1], f32, tag="c2s", name="c2s")
                    nc.scalar.activation(c2s[:], c2p[:], AF.Identity, bias=nb2[:])
                    nc.sync.dma_start(out=outc_d, in_=c2s[:])

            # ---------- node outputs (from final h_loc = h_loc[L % 2]) ----------
            hfin = h_loc[L % 2]
            nc.sync.dma_start(out=outh_d.rearrange("(t p) c -> p t c", p=128),
                              in_=hfin[:].rearrange("p (t c) -> p t c", t=RT))
            hTfin = pp.tile([128, R], f32)
            for rt in range(RT):
                tp = pss.tile([128, 128], f32, tag="ps_small", name=f"tpf_{rt}")
                nc.tensor.transpose(tp[:], hfin[:, rt * H:(rt + 1) * H], ident[:])
                nc.scalar.copy(hTfin[:, rt * 128:(rt + 1) * 128], tp[:])
            z1p = pss.tile([H // 2, R], f32, tag="ps_small", name="z1p")
            nc.tensor.matmul(z1p[:], cW1[:], hTfin[:], start=True, stop=True)
            z1s = wp.tile([H // 2, R], f32, tag="z1s", name="z1s")
            nc.scalar.activation(z1s[:], z1p[:], AF.Relu, bias=cb1[:])
            lgp = pss.tile([CDIM, R], f32, tag="ps_small", name="lgp")
            nc.tensor.matmul(lgp[:], cW2[:], z1s[:], start=True, stop=True)
            lgs = wp.tile([CDIM, R], f32, tag="lgs", name="lgs")
            nc.scalar.activation(lgs[:], lgp[:], AF.Identity, bias=cb2[:])
            nc.sync.dma_start(out=logT_d, in_=lgs[:])

    nc.compile()
    return nc


def _install_ntff_hook():
    """The image's antenv lacks axon_hooks; inject it so trace=True works."""
    import sys
    import types

    try:
        from antenv.axon_hooks import get_axon_ntff_profile_hook  # noqa: F401
        return
    except ImportError:
        pass
    import antenv
    mod = types.ModuleType("antenv.axon_hooks")
    state = {"hook": None}
    mod.set_axon_ntff_profile_hook = lambda h: state.__setitem__("hook", h)
    mod.get_axon_ntff_profile_hook = lambda: state["hook"]
    sys.modules["antenv.axon_hooks"] = mod
    antenv.axon_hooks = mod
    try:
        from trn_agent_boot.trn_boot import _ntff_profile_via_ctypes
        mod.set_axon_ntff_profile_hook(
            _ntff_profile_via_ctypes("/opt/axon/libaxon_pjrt.so"))
    except Exception:
        pass


def _host_prep(inputs):
    """Shard/layout the full inputs per core (pure layout, no arithmetic)."""
    I = {k: np.ascontiguousarray(np.asarray(v, dtype=np.float32))
         for k, v in inputs.items()}
    adj = I["adj"]
    x = I["x"]
    xT = np.ascontiguousarray(x.T)
    adjTf = np.ascontiguousarray(adj.T)  # [j, i]
    attn_W = I["attn_W"]
    shared = {
        "xT": xT,
        "enc_W": I["enc_W"], "enc_b": I["enc_b"],
        "gcn_W": I["gcn_W"], "gcn_b": I["gcn_b"],
        "attn_Wcat": np.ascontiguousarray(
            attn_W.transpose(0, 2, 1, 3).reshape(L, H, H)),
        "attn_WT": np.ascontiguousarray(attn_W.transpose(0, 1, 3, 2)),
        "attn_Wb": I["attn_Wb"], "attn_a": I["attn_a"], "attn_ab": I["attn_ab"],
        "cls_W1": I["cls_W1"], "cls_b1": I["cls_b1"],
        "cls_W2": I["cls_W2"], "cls_b2": I["cls_b2"],
        "con_W1": I["con_W1"], "con_b1": I["con_b1"],
        "con_W2": I["con_W2"], "con_b2": I["con_b2"],
    }
    in_maps = []
    for c in range(NCORES):
        blk = adjTf[:, c * R:(c + 1) * R]  # [4096, 512]
        adjT_dev = np.ascontiguousarray(
            blk.reshape(NT, 128, R).transpose(1, 0, 2).reshape(128, NT * R))
        m = dict(shared)
        m["adjT"] = adjT_dev
        m["xT_loc"] = np.ascontiguousarray(xT[:, c * R:(c + 1) * R])
        in_maps.append(m)
    return in_maps


def run(inputs, trace=False):
    from concourse import bass_utils
    _install_ntff_hook()
    if "nc" not in _CACHE:
        _CACHE["nc"] = _build_nc()
    nc = _CACHE["nc"]
    in_maps = _host_prep(inputs)
    res = bass_utils.run_bass_kernel_spmd(
        nc, in_maps, core_ids=list(range(NCORES)), trace=trace)
    node_logits = np.concatenate(
        [np.ascontiguousarray(res.results[c]["logitsT"].T) for c in range(NCORES)],
        axis=0)
    h = np.concatenate([res.results[c]["out_h"] for c in range(NCORES)], axis=0)
    contagion = res.results[0]["out_con"]
    return (node_logits, h, contagion), res


def kernel(**inputs):
    (node_logits, h, contagion), _ = run(inputs, trace=False)
    return (node_logits.astype(np.float32), h.astype(np.float32),
            contagion.astype(np.float32))


# revision 10
# speedup vs baseline: 1.2136x; 1.0582x over previous
"""GNN message-passing kernel for 8 TRN2 NeuronCores.

Reference: 3 layers of (GCN + 4-head graph attention) over a 4096x4096
adjacency, plus encoder / classifier / contagion heads.

Algebraic reformulation of the attention (validated to ~1e-6 vs the
reference): non-edges get score exactly 0 (exp=1), so with u = exp(t),
es_i = exp(s_i + ab):

    probs @ hh = [es * (adj @ (u*hh)) + (S_all - adj @ hh)] / Z
    Z          = es * (adj @ u) + (N - deg)

Everything reduces to row-blocks of adj @ M where M is [N, 388]:
    cols [0:128)    dinv * h          (GCN, symmetric-normalized)
    cols [128:256)  u * hh            (4 heads x 32)
    cols [256:384)  -hh               (negated; ones x S_all added on top)
    cols [384:388)  u                 (4 heads)

Sharding: core c owns rows [512c, 512c+512). Its stationary operand is
adj[rows_c, :].T laid out k-major in SBUF as bf16 (adj is 0/1 -> exact).
M is split hi/lo into two bf16 matmuls accumulated in the same PSUM bank
(~1.5e-5 relative precision). h (N x 128) is all-gathered each layer.
"""

import numpy as np

NCORES = 8
N = 4096
F = 64
H = 128
NH = 4
HD = 32
L = 3
CDIM = 7
R = N // NCORES          # 512 rows per core
NT = N // 128            # 32 j-tiles
RT = R // 128            # 4 r-tiles per core
MW = H + H + H + NH      # 388 columns of M
USE_SPLIT = False        # bf16-only M measured at 4.6e-4 rel err (gate 2e-2)

_CACHE = {}


def _build_nc():
    import concourse.bass as bass
    import concourse.mybir as mybir
    from concourse import bacc
    import concourse.tile as tile
    from concourse.masks import make_identity

    dt = mybir.dt
    f32 = dt.float32
    bf16 = dt.bfloat16
    AF = mybir.ActivationFunctionType

    nc = bacc.Bacc("TRN2", target_bir_lowering=False, debug=False,
                   num_devices=NCORES)

    # ---- I/O ----
    adjT_d = nc.dram_tensor("adjT", [128, NT * R], f32, kind="ExternalInput").ap()
    xT_d = nc.dram_tensor("xT", [F, N], f32, kind="ExternalInput").ap()
    xTl_d = nc.dram_tensor("xT_loc", [F, R], f32, kind="ExternalInput").ap()
    encW_d = nc.dram_tensor("enc_W", [F, H], f32, kind="ExternalInput").ap()
    encb_d = nc.dram_tensor("enc_b", [H], f32, kind="ExternalInput").ap()
    gcnW_d = nc.dram_tensor("gcn_W", [L, H, H], f32, kind="ExternalInput").ap()
    gcnb_d = nc.dram_tensor("gcn_b", [L, H], f32, kind="ExternalInput").ap()
    awcat_d = nc.dram_tensor("attn_Wcat", [L, H, H], f32, kind="ExternalInput").ap()
    awt_d = nc.dram_tensor("attn_WT", [L, NH, HD, H], f32, kind="ExternalInput").ap()
    awb_d = nc.dram_tensor("attn_Wb", [L, NH, HD], f32, kind="ExternalInput").ap()
    aa_d = nc.dram_tensor("attn_a", [L, NH, 2 * HD], f32, kind="ExternalInput").ap()
    aab_d = nc.dram_tensor("attn_ab", [L, NH], f32, kind="ExternalInput").ap()
    cW1_d = nc.dram_tensor("cls_W1", [H, H // 2], f32, kind="ExternalInput").ap()
    cb1_d = nc.dram_tensor("cls_b1", [H // 2], f32, kind="ExternalInput").ap()
    cW2_d = nc.dram_tensor("cls_W2", [H // 2, CDIM], f32, kind="ExternalInput").ap()
    cb2_d = nc.dram_tensor("cls_b2", [CDIM], f32, kind="ExternalInput").ap()
    nW1_d = nc.dram_tensor("con_W1", [H, H // 2], f32, kind="ExternalInput").ap()
    nb1_d = nc.dram_tensor("con_b1", [H // 2], f32, kind="ExternalInput").ap()
    nW2_d = nc.dram_tensor("con_W2", [H // 2, 1], f32, kind="ExternalInput").ap()
    nb2_d = nc.dram_tensor("con_b2", [1], f32, kind="ExternalInput").ap()

    logT_d = nc.dram_tensor("logitsT", [CDIM, R], f32, kind="ExternalOutput").ap()
    outh_d = nc.dram_tensor("out_h", [R, H], f32, kind="ExternalOutput").ap()
    outc_d = nc.dram_tensor("out_con", [1, 1], f32, kind="ExternalOutput").ap()

    mult = mybir.AluOpType.mult
    add = mybir.AluOpType.add

    with tile.TileContext(nc) as tc:
        with (
            tc.tile_pool(name="persist", bufs=1) as pp,
            tc.tile_pool(name="chunk", bufs=2) as chp,
            tc.tile_pool(name="mpool", bufs=34) as mp,
            tc.tile_pool(name="work", bufs=3) as wp,
            tc.tile_pool(name="ps_y", bufs=4, space="PSUM") as psy,
            tc.tile_pool(name="ps_s", bufs=4, space="PSUM") as pss,
            tc.tile_pool(name="dram", bufs=1, space="DRAM") as dp,
        ):
            # ---------- constants ----------
            ident = pp.tile([128, 128], f32)
            make_identity(nc, ident)
            identb = pp.tile([128, 128], bf16)
            nc.vector.tensor_copy(identb[:], ident[:])
            ones_row = pp.tile([1, 128], bf16)
            nc.vector.memset(ones_row[:], 1.0)
            ones_rowf = pp.tile([1, 128], f32)
            nc.vector.memset(ones_rowf[:], 1.0)
            ncell = pp.tile([1, 1], f32)
            nc.vector.memset(ncell[:], float(N))
            ones_colb = pp.tile([128, 1], bf16)
            nc.vector.memset(ones_colb[:], 1.0)
            ones_colf = pp.tile([128, 1], f32)
            nc.vector.memset(ones_colf[:], 1.0)

            # ---------- small weights ----------
            encWf = pp.tile([F, H], f32)
            nc.sync.dma_start(out=encWf[:], in_=encW_d)
            encW = pp.tile([F, H], bf16)
            nc.vector.tensor_copy(encW[:], encWf[:])
            encbf = pp.tile([1, H], f32)
            nc.sync.dma_start(out=encbf[:], in_=encb_d.unsqueeze(0))
            encb = pp.tile([1, H], bf16)
            nc.vector.tensor_copy(encb[:], encbf[:])
            xTf = pp.tile([F, N], f32)
            for i in range(4):
                nc.sync.dma_start(out=xTf[:, i * 1024:(i + 1) * 1024],
                                  in_=xT_d[:, i * 1024:(i + 1) * 1024])
            xT = pp.tile([F, N], bf16)
            nc.vector.tensor_copy(xT[:], xTf[:])
            xTlf = pp.tile([F, R], f32)
            nc.sync.dma_start(out=xTlf[:], in_=xTl_d)
            xTl = pp.tile([F, R], bf16)
            nc.vector.tensor_copy(xTl[:], xTlf[:])

            gcnW = pp.tile([128, L * H], f32)
            gcnbf = pp.tile([1, L * H], f32)
            for l in range(L):
                nc.sync.dma_start(out=gcnW[:, l * H:(l + 1) * H], in_=gcnW_d[l])
                nc.sync.dma_start(out=gcnbf[:, l * H:(l + 1) * H],
                                  in_=gcnb_d[l].unsqueeze(0))
            gcnb = pp.tile([1, L * H], bf16)
            nc.vector.tensor_copy(gcnb[:], gcnbf[:])
            cW1 = pp.tile([H, H // 2], f32)
            nc.sync.dma_start(out=cW1[:], in_=cW1_d)
            cb1 = pp.tile([H // 2, 1], f32)
            nc.sync.dma_start(out=cb1[:], in_=cb1_d.unsqueeze(1))
            cW2 = pp.tile([H // 2, CDIM], f32)
            nc.sync.dma_start(out=cW2[:], in_=cW2_d)
            cb2 = pp.tile([CDIM, 1], f32)
            nc.sync.dma_start(out=cb2[:], in_=cb2_d.unsqueeze(1))
            nW1 = pp.tile([H, H // 2], f32)
            nc.sync.dma_start(out=nW1[:], in_=nW1_d)
            nb1 = pp.tile([H // 2, 1], f32)
            nc.sync.dma_start(out=nb1[:], in_=nb1_d.unsqueeze(1))
            nW2 = pp.tile([H // 2, 1], f32)
            nc.sync.dma_start(out=nW2[:], in_=nW2_d)
            nb2 = pp.tile([1, 1], f32)
            nc.sync.dma_start(out=nb2[:], in_=nb2_d.unsqueeze(0))
            ab_sb = pp.tile([1, L * NH], f32)
            nc.sync.dma_start(out=ab_sb[:], in_=aab_d.flatten().unsqueeze(0))

            # per-(l,h) columns used to fold s/t into the hh matmul
            awt_sb = pp.tile([HD, L * NH * H], f32)
            adcol = pp.tile([HD, L * NH], f32)   # a_dst columns
            ascol = pp.tile([HD, L * NH], f32)   # a_src columns
            wbcol = pp.tile([HD, L * NH], f32)   # attn_Wb columns
            for l in range(L):
                for hh_ in range(NH):
                    k = l * NH + hh_
                    nc.sync.dma_start(out=awt_sb[:, k * H:(k + 1) * H],
                                      in_=awt_d[l, hh_])
                    nc.sync.dma_start(out=adcol[:, k:k + 1],
                                      in_=aa_d[l, hh_, HD:].unsqueeze(1))
                    nc.sync.dma_start(out=ascol[:, k:k + 1],
                                      in_=aa_d[l, hh_, 0:HD].unsqueeze(1))
                    nc.sync.dma_start(out=wbcol[:, k:k + 1],
                                      in_=awb_d[l, hh_].unsqueeze(1))

            # W_big[l] = [Wcat (128) | w_t (4) | w_s (4)]  and consts rows
            WB = H + 2 * NH  # 136
            wbig = pp.tile([128, L * WB], f32)
            wbigb = pp.tile([128, L * WB], bf16)
            consts = pp.tile([1, L * WB], f32)
            constsb = pp.tile([1, L * WB], bf16)
            for l in range(L):
                nc.sync.dma_start(out=wbig[:, l * WB:l * WB + H], in_=awcat_d[l])
                nc.sync.dma_start(out=consts[:, l * WB:l * WB + H],
                                  in_=awb_d[l].flatten().unsqueeze(0))
                for hh_ in range(NH):
                    k = l * NH + hh_
                    wt_ps = pss.tile([128, 1], f32, tag="ps_small", name=f"wt_{l}_{hh_}")
                    nc.tensor.matmul(wt_ps[:], awt_sb[:, k * H:(k + 1) * H],
                                     adcol[:, k:k + 1], start=True, stop=True)
                    nc.vector.tensor_copy(wbig[:, l * WB + H + hh_:l * WB + H + hh_ + 1],
                                          wt_ps[:])
                    ws_ps = pss.tile([128, 1], f32, tag="ps_small", name=f"ws_{l}_{hh_}")
                    nc.tensor.matmul(ws_ps[:], awt_sb[:, k * H:(k + 1) * H],
                                     ascol[:, k:k + 1], start=True, stop=True)
                    nc.vector.tensor_copy(
                        wbig[:, l * WB + H + NH + hh_:l * WB + H + NH + hh_ + 1],
                        ws_ps[:])
                    ct_ps = pss.tile([1, 1], f32, tag="ps_small", name=f"ct_{l}_{hh_}")
                    nc.tensor.matmul(ct_ps[:], wbcol[:HD, k:k + 1],
                                     adcol[:, k:k + 1], start=True, stop=True)
                    nc.vector.tensor_copy(consts[:, l * WB + H + hh_:l * WB + H + hh_ + 1],
                                          ct_ps[:])
                    cs_ps = pss.tile([1, 1], f32, tag="ps_small", name=f"cs_{l}_{hh_}")
                    nc.tensor.matmul(cs_ps[:], wbcol[:HD, k:k + 1],
                                     ascol[:, k:k + 1], start=True, stop=True)
                    # c_s + ab  (bias is per-partition [1,1])
                    nc.scalar.activation(
                        consts[:, l * WB + H + NH + hh_:l * WB + H + NH + hh_ + 1],
                        cs_ps[:], AF.Identity, bias=ab_sb[:, k:k + 1])

            nc.vector.tensor_copy(wbigb[:], wbig[:])
            nc.vector.tensor_copy(constsb[:], consts[:])

            # ---------- xT + encoder (h0 full and local) ----------
            # ---------- adjT load + bf16 cast (cast on idle GpSimd) ----------
            adjT = pp.tile([128, NT * R], bf16)
            NCHUNK = 16
            CW = NT * R // NCHUNK  # 1024 cols per chunk
            for i in range(NCHUNK):
                ach = chp.tile([128, CW], f32, tag="ach", name=f"ach_{i}")
                nc.sync.dma_start(out=ach[:], in_=adjT_d[:, i * CW:(i + 1) * CW])
                if i % 2 == 0:
                    nc.scalar.copy(adjT[:, i * CW:(i + 1) * CW], ach[:])
                else:
                    nc.vector.tensor_copy(adjT[:, i * CW:(i + 1) * CW], ach[:])

            h_full = pp.tile([128, NT * H], bf16)
            h_loc = [pp.tile([128, RT * H], f32, name=f"h_loc_{i}") for i in range(2)]
            hT_full = pp.tile([128, NT * H], bf16)
            hT_loc = pp.tile([128, R], bf16)

            for jt in range(NT):
                h0p = pss.tile([128, H], f32, tag="ps_small", name=f"h0p_{jt}")
                nc.tensor.matmul(h0p[:], xT[:, jt * 128:(jt + 1) * 128], encW[:],
                                 start=True, stop=False)
                nc.tensor.matmul(h0p[:], ones_row[:], encb[:], start=False, stop=True)
                nc.scalar.activation(h_full[:, jt * H:(jt + 1) * H], h0p[:], AF.Relu)
            for rt in range(RT):
                h0p = pss.tile([128, H], f32, tag="ps_small", name=f"h0pl_{rt}")
                nc.tensor.matmul(h0p[:], xTl[:, rt * 128:(rt + 1) * 128], encW[:],
                                 start=True, stop=False)
                nc.tensor.matmul(h0p[:], ones_row[:], encb[:], start=False, stop=True)
                nc.scalar.activation(h_loc[0][:, rt * H:(rt + 1) * H], h0p[:], AF.Relu)

            # ---------- deg (local rows) + allgather -> deg_full ----------
            deg_loc = pp.tile([128, RT], f32)
            for rt in range(RT):
                dps = pss.tile([128, 1], f32, tag="ps_small", name=f"degp_{rt}")
                for jt in range(NT):
                    nc.tensor.matmul(dps[:], adjT[:, jt * R + rt * 128:jt * R + (rt + 1) * 128],
                                     ones_colb[:], start=(jt == 0), stop=(jt == NT - 1))
                nc.vector.tensor_copy(deg_loc[:, rt:rt + 1], dps[:])
            cc_dego = dp.tile([N, 1], f32, addr_space="Shared")
            cc_degi = dp.tile([R, 1], f32)
            nc.sync.dma_start(out=cc_degi.rearrange("(t p) o -> p t o", p=128),
                              in_=deg_loc[:].unsqueeze(2))
            nc.gpsimd.collective_compute(
                "AllGather", mybir.AluOpType.bypass,
                replica_groups=[list(range(NCORES))],
                ins=[cc_degi.opt()], outs=[cc_dego.opt()])
            deg_full = pp.tile([128, NT], f32)
            nc.sync.dma_start(out=deg_full[:],
                              in_=cc_dego.rearrange("(t p) o -> p (t o)", p=128))

            # dinv = 1/sqrt(deg+1); Ndeg = N - deg
            dinv_full = pp.tile([128, NT], f32)
            nc.scalar.activation(dinv_full[:], deg_full[:], AF.Sqrt, bias=1.0)
            nc.vector.reciprocal(dinv_full[:], dinv_full[:])
            dinv_loc = pp.tile([128, RT], f32)
            nc.scalar.activation(dinv_loc[:], deg_loc[:], AF.Sqrt, bias=1.0)
            nc.vector.reciprocal(dinv_loc[:], dinv_loc[:])
            dinv2_loc = pp.tile([128, RT], f32)
            nc.vector.tensor_tensor(dinv2_loc[:], dinv_loc[:], dinv_loc[:], op=mult)
            ndeg_loc = pp.tile([128, RT], f32)
            nc.vector.tensor_scalar(ndeg_loc[:], deg_loc[:], -1.0, float(N),
                                    op0=mult, op1=add)

            # ---------- layers ----------
            for l in range(L):
                hf = h_full
                hl = h_loc[l % 2]
                hl_new = h_loc[(l + 1) % 2]
                wb_l = wbig[:, l * WB:(l + 1) * WB]
                co_l = consts[:, l * WB:(l + 1) * WB]
                wbb_l = wbigb[:, l * WB:(l + 1) * WB]
                cob_l = constsb[:, l * WB:(l + 1) * WB]

                # local transpose of h_loc for s/t (bf16 for the bf16 hh matmul)
                hlb = wp.tile([128, RT * H], bf16, tag="hlb", name=f"hlb_{l}")
                nc.vector.tensor_copy(hlb[:], hl[:])
                for rt in range(RT):
                    tp = pss.tile([128, 128], bf16, tag="ps_small", name=f"tp_{l}_{rt}")
                    nc.tensor.transpose(tp[:], hlb[:, rt * H:(rt + 1) * H], identb[:])
                    nc.vector.tensor_copy(hT_loc[:, rt * 128:(rt + 1) * 128], tp[:])

                # hT for this layer: PE transposes, hoisted so the per-jt
                # hh chain runs against ready hT tiles; accum_out builds the
                # per-feature running sum for S_all on the way out
                hacc = wp.tile([128, NT], f32, tag="hacc", name=f"hacc_{l}")
                for jt in range(NT):
                    tp = pss.tile([128, 128], bf16, tag="ps_small",
                                  name=f"tph_{l}_{jt}")
                    nc.tensor.transpose(tp[:], hf[:, jt * H:(jt + 1) * H], identb[:])
                    nc.vector.tensor_scalar(hT_full[:, jt * H:(jt + 1) * H], tp[:],
                                            1.0, 0.0, op0=mult,
                                            op1=mybir.AluOpType.add,
                                            accum_out=hacc[:, jt:jt + 1])

                # Y accumulators (4 r-tiles x 388 cols)
                ybank = [psy.tile([128, MW], f32, tag="y", name=f"y_{l}_{rt}")
                         for rt in range(RT)]

                mtiles = []
                for jt in range(NT):
                    # hh/t/s for this j-tile (bf16 matmul, f32 accumulate)
                    hhp = pss.tile([128, WB], f32, tag="ps_small", name=f"hhp_{l}_{jt}")
                    nc.tensor.matmul(hhp[:], hT_full[:, jt * H:(jt + 1) * H], wbb_l,
                                     start=True, stop=False)
                    nc.tensor.matmul(hhp[:], ones_row[:], cob_l, start=False, stop=True)
                    # assemble M tile (bf16)
                    mhi = mp.tile([128, MW], bf16, tag="mhi", name=f"mhi_{l}_{jt}")
                    mtiles.append(mhi)
                    # u = exp(t)
                    nc.scalar.activation(mhi[:, 384:388], hhp[:, H:H + NH], AF.Exp)
                    # u * hh (broadcast u over the 32 cols of each head)
                    ub = mhi[:, 384:388].unsqueeze(2).broadcast_to([128, NH, HD])
                    nc.vector.tensor_tensor(
                        mhi[:, H:2 * H].rearrange("p (h d) -> p h d", h=NH),
                        hhp[:, 0:H].rearrange("p (h d) -> p h d", h=NH), ub, op=mult)
                    # -hh
                    nc.vector.tensor_scalar_mul(mhi[:, 2 * H:3 * H], hhp[:, 0:H], -1.0)
                    if l == 0:
                        # dinv (deg allgather) may not be ready yet: do the
                        # independent columns now, h' columns in a second pass
                        for rt in range(RT):
                            lhs = adjT[:, jt * R + rt * 128:jt * R + (rt + 1) * 128]
                            nc.tensor.matmul(ybank[rt][:, H:], lhs, mhi[:, H:],
                                             start=(jt == 0), stop=False)
                    else:
                        # h' = dinv * h
                        nc.vector.tensor_scalar_mul(
                            mhi[:, 0:H], hf[:, jt * H:(jt + 1) * H],
                            dinv_full[:, jt:jt + 1])
                        for rt in range(RT):
                            lhs = adjT[:, jt * R + rt * 128:jt * R + (rt + 1) * 128]
                            nc.tensor.matmul(ybank[rt][:], lhs, mhi[:],
                                             start=(jt == 0), stop=False)
                if l == 0:
                    for jt in range(NT):
                        mhi = mtiles[jt]
                        nc.vector.tensor_scalar_mul(
                            mhi[:, 0:H], hf[:, jt * H:(jt + 1) * H],
                            dinv_full[:, jt:jt + 1])
                        for rt in range(RT):
                            lhs = adjT[:, jt * R + rt * 128:jt * R + (rt + 1) * 128]
                            nc.tensor.matmul(ybank[rt][:, 0:H], lhs, mhi[:, 0:H],
                                             start=(jt == 0), stop=False)

                # S_row388 = [0 | 0 | S_all | 0]
                hsum = wp.tile([128, 1], f32, tag="hsum", name=f"hsum_{l}")
                nc.vector.tensor_reduce(out=hsum[:], in_=hacc[:],
                                        op=add, axis=mybir.AxisListType.X)
                sraw = pss.tile([1, WB], f32, tag="ps_small", name=f"sraw_{l}")
                nc.tensor.matmul(sraw[:], hsum[:], wb_l, start=True, stop=False)
                nc.tensor.matmul(sraw[:], ncell[:], co_l, start=False, stop=True)
                srow = wp.tile([1, MW], f32, tag="srow", name=f"srow_{l}")
                nc.vector.memset(srow[:], 0.0)
                nc.vector.tensor_copy(srow[:, 2 * H:3 * H], sraw[:, 0:H])
                srhi = wp.tile([1, MW], bf16, tag="srhi", name=f"srhi_{l}")
                nc.vector.tensor_copy(srhi[:], srow[:])
                srlo = wp.tile([1, MW], bf16, tag="srlo", name=f"srlo_{l}")
                nc.vector.tensor_tensor(srlo[:], srow[:], srhi[:],
                                        op=mybir.AluOpType.subtract)
                for rt in range(RT):
                    nc.tensor.matmul(ybank[rt][:], ones_row[:], srhi[:],
                                     start=False, stop=False)
                    nc.tensor.matmul(ybank[rt][:], ones_row[:], srlo[:],
                                     start=False, stop=True)

                # epilogue per r-tile
                for rt in range(RT):
                    y = ybank[rt]
                    # s/t for local rows; es = exp(s + c_s + ab)
                    stp = pss.tile([128, 2 * NH], f32, tag="ps_small",
                                   name=f"stp_{l}_{rt}")
                    nc.tensor.matmul(stp[:], hT_loc[:, rt * 128:(rt + 1) * 128],
                                     wbb_l[:, H:], start=True, stop=False)
                    nc.tensor.matmul(stp[:], ones_row[:], cob_l[:, H:],
                                     start=False, stop=True)
                    es = wp.tile([128, 2 * NH], f32, tag="es", name=f"es_{l}_{rt}")
                    nc.scalar.activation(es[:], stp[:], AF.Exp)
                    # attention numerator: es*P + (S_all - Q)
                    sq = wp.tile([128, H], f32, tag="sq", name=f"sq_{l}_{rt}")
                    nc.scalar.copy(sq[:], y[:, 2 * H:3 * H])
                    pre = wp.tile([128, H], f32, tag="pre", name=f"pre_{l}_{rt}")
                    for hh_ in range(NH):
                        nc.vector.scalar_tensor_tensor(
                            pre[:, hh_ * HD:(hh_ + 1) * HD],
                            y[:, H + hh_ * HD:H + (hh_ + 1) * HD],
                            es[:, NH + hh_:NH + hh_ + 1],
                            sq[:, hh_ * HD:(hh_ + 1) * HD],
                            op0=mult, op1=add)
                    # Z = es*AU + (N - deg); rec = 1/Z
                    zt = wp.tile([128, NH], f32, tag="zt", name=f"zt_{l}_{rt}")
                    nc.vector.tensor_tensor(zt[:], y[:, 3 * H:3 * H + NH],
                                            es[:, NH:2 * NH], op=mult)
                    nc.vector.tensor_scalar_add(zt[:], zt[:],
                                                ndeg_loc[:, rt:rt + 1])
                    nc.vector.reciprocal(zt[:], zt[:])
                    # GCN: sup = dinv*(A1 + h'_loc) = dinv*A1 + dinv^2*h_loc
                    hp2 = wp.tile([128, H], f32, tag="hp2", name=f"hp2_{l}_{rt}")
                    nc.vector.tensor_scalar_mul(hp2[:], hl[:, rt * H:(rt + 1) * H],
                                                dinv2_loc[:, rt:rt + 1])
                    sup = wp.tile([128, H], f32, tag="sup", name=f"sup_{l}_{rt}")
                    nc.vector.scalar_tensor_tensor(sup[:], y[:, 0:H],
                                                   dinv_loc[:, rt:rt + 1], hp2[:],
                                                   op0=mult, op1=add)
                    supt = pss.tile([128, 128], f32, tag="ps_small",
                                    name=f"supt_{l}_{rt}")
                    nc.tensor.transpose(supt[:], sup[:], ident[:])
                    supts = wp.tile([128, 128], f32, tag="supts", name=f"supts_{l}_{rt}")
                    nc.scalar.copy(supts[:], supt[:])
                    gcnp = pss.tile([128, H], f32, tag="ps_small", name=f"gcnp_{l}_{rt}")
                    nc.tensor.matmul(gcnp[:], supts[:], gcnW[:, l * H:(l + 1) * H],
                                     start=True, stop=False)
                    nc.tensor.matmul(gcnp[:], ones_row[:], gcnb[:, l * H:(l + 1) * H],
                                     start=False, stop=True)
                    hg = wp.tile([128, H], f32, tag="hg", name=f"hg_{l}_{rt}")
                    nc.scalar.activation(hg[:], gcnp[:], AF.Relu)
                    # h_new = relu(hg + pre * rec)
                    for hh_ in range(NH):
                        nc.vector.scalar_tensor_tensor(
                            pre[:, hh_ * HD:(hh_ + 1) * HD],
                            pre[:, hh_ * HD:(hh_ + 1) * HD],
                            zt[:, hh_:hh_ + 1],
                            hg[:, hh_ * HD:(hh_ + 1) * HD], op0=mult, op1=add)
                    nc.scalar.activation(hl_new[:, rt * H:(rt + 1) * H], pre[:],
                                         AF.Relu)

                if l < L - 1:
                    # allgather h in bf16 (halves collective + reload traffic)
                    hlnb = wp.tile([128, RT * H], bf16, tag="hlnb", name=f"hlnb_{l}")
                    nc.vector.tensor_copy(hlnb[:], hl_new[:])
                    cci = dp.tile([R, H], bf16, name=f"cci_{l}")
                    cco = dp.tile([N, H], bf16, addr_space="Shared", name=f"cco_{l}")
                    nc.sync.dma_start(out=cci.rearrange("(t p) c -> p t c", p=128),
                                      in_=hlnb[:].rearrange("p (t c) -> p t c", t=RT))
                    nc.gpsimd.collective_compute(
                        "AllGather", mybir.AluOpType.bypass,
                        replica_groups=[list(range(NCORES))],
                        ins=[cci.opt()], outs=[cco.opt()])
                    ccot = cco.rearrange("(t p) c -> p t c", p=128)
                    for i in range(8):
                        nc.sync.dma_start(
                            out=h_full[:, i * 4 * H:(i + 1) * 4 * H]
                                .rearrange("p (t c) -> p t c", t=4),
                            in_=ccot[:, i * 4:(i + 1) * 4, :])

                if l == L - 1:
                    # contagion: local column-sum -> tiny allgather -> mean MLP
                    meanp = pss.tile([128, 1], f32, tag="ps_small", name="meanp")
                    for rt in range(RT):
                        nc.tensor.matmul(meanp[:], hl_new[:, rt * H:(rt + 1) * H],
                                         ones_colf[:], start=(rt == 0),
                                         stop=(rt == RT - 1))
                    mloc = wp.tile([128, 1], f32, tag="mloc", name="mloc")
                    nc.vector.tensor_copy(mloc[:], meanp[:])
                    mgi = dp.tile([128, 1], f32, name="mgi")
                    mgo = dp.tile([8 * 128, 1], f32, addr_space="Shared", name="mgo")
                    nc.sync.dma_start(out=mgi[:], in_=mloc[:])
                    nc.gpsimd.collective_compute(
                        "AllGather", mybir.AluOpType.bypass,
                        replica_groups=[list(range(NCORES))],
                        ins=[mgi.opt()], outs=[mgo.opt()])
                    mall = wp.tile([128, 8], f32, tag="mall", name="mall")
                    nc.sync.dma_start(out=mall[:],
                                      in_=mgo.rearrange("(c p) u -> p (c u)", p=128))
                    msum = wp.tile([128, 1], f32, tag="msum", name="msum")
                    nc.vector.tensor_reduce(out=msum[:], in_=mall[:],
                                            op=add, axis=mybir.AxisListType.X)
                    means = wp.tile([128, 1], f32, tag="means", name="means")
                    nc.scalar.mul(means[:], msum[:], 1.0 / N)
                    c1p = pss.tile([H // 2, 1], f32, tag="ps_small", name="c1p")
                    nc.tensor.matmul(c1p[:], nW1[:], means[:], start=True, stop=True)
                    c1s = wp.tile([H // 2, 1], f32, tag="c1s", name="c1s")
                    nc.scalar.activation(c1s[:], c1p[:], AF.Relu, bias=nb1[:])
                    c2p = pss.tile([1, 1], f32, tag="ps_small", name="c2p")
                    nc.tensor.matmul(c2p[:], nW2[:], c1s[:], start=True, stop=True)
                    c2s = wp.tile([1, 1], f32, tag="c2s", name="c2s")
                    nc.scalar.activation(c2s[:], c2p[:], AF.Identity, bias=nb2[:])
                    nc.sync.dma_start(out=outc_d, in_=c2s[:])

            # ---------- node outputs (from final h_loc = h_loc[L % 2]) ----------
            hfin = h_loc[L % 2]
            nc.sync.dma_start(out=outh_d.rearrange("(t p) c -> p t c", p=128),
                              in_=hfin[:].rearrange("p (t c) -> p t c", t=RT))
            hTfin = pp.tile([128, R], f32)
            for rt in range(RT):
                tp = pss.tile([128, 128], f32, tag="ps_small", name=f"tpf_{rt}")
                nc.tensor.transpose(tp[:], hfin[:, rt * H:(rt + 1) * H], ident[:])
                nc.scalar.copy(hTfin[:, rt * 128:(rt + 1) * 128], tp[:])
            z1p = pss.tile([H // 2, R], f32, tag="ps_small", name="z1p")
            nc.tensor.matmul(z1p[:], cW1[:], hTfin[:], start=True, stop=True)
            z1s = wp.tile([H // 2, R], f32, tag="z1s", name="z1s")
            nc.scalar.activation(z1s[:], z1p[:], AF.Relu, bias=cb1[:])
            lgp = pss.tile([CDIM, R], f32, tag="ps_small", name="lgp")
            nc.tensor.matmul(lgp[:], cW2[:], z1s[:], start=True, stop=True)
            lgs = wp.tile([CDIM, R], f32, tag="lgs", name="lgs")
            nc.scalar.activation(lgs[:], lgp[:], AF.Identity, bias=cb2[:])
            nc.sync.dma_start(out=logT_d, in_=lgs[:])

    nc.compile()
    return nc


def _install_ntff_hook():
    """The image's antenv lacks axon_hooks; inject it so trace=True works."""
    import sys
    import types

    try:
        from antenv.axon_hooks import get_axon_ntff_profile_hook  # noqa: F401
        return
    except ImportError:
        pass
    import antenv
    mod = types.ModuleType("antenv.axon_hooks")
    state = {"hook": None}
    mod.set_axon_ntff_profile_hook = lambda h: state.__setitem__("hook", h)
    mod.get_axon_ntff_profile_hook = lambda: state["hook"]
    sys.modules["antenv.axon_hooks"] = mod
    antenv.axon_hooks = mod
    try:
        from trn_agent_boot.trn_boot import _ntff_profile_via_ctypes
        mod.set_axon_ntff_profile_hook(
            _ntff_profile_via_ctypes("/opt/axon/libaxon_pjrt.so"))
    except Exception:
        pass


def _host_prep(inputs):
    """Shard/layout the full inputs per core (pure layout, no arithmetic)."""
    I = {k: np.ascontiguousarray(np.asarray(v, dtype=np.float32))
         for k, v in inputs.items()}
    adj = I["adj"]
    x = I["x"]
    xT = np.ascontiguousarray(x.T)
    adjTf = np.ascontiguousarray(adj.T)  # [j, i]
    attn_W = I["attn_W"]
    shared = {
        "xT": xT,
        "enc_W": I["enc_W"], "enc_b": I["enc_b"],
        "gcn_W": I["gcn_W"], "gcn_b": I["gcn_b"],
        "attn_Wcat": np.ascontiguousarray(
            attn_W.transpose(0, 2, 1, 3).reshape(L, H, H)),
        "attn_WT": np.ascontiguousarray(attn_W.transpose(0, 1, 3, 2)),
        "attn_Wb": I["attn_Wb"], "attn_a": I["attn_a"], "attn_ab": I["attn_ab"],
        "cls_W1": I["cls_W1"], "cls_b1": I["cls_b1"],
        "cls_W2": I["cls_W2"], "cls_b2": I["cls_b2"],
        "con_W1": I["con_W1"], "con_b1": I["con_b1"],
        "con_W2": I["con_W2"], "con_b2": I["con_b2"],
    }
    in_maps = []
    for c in range(NCORES):
        blk = adjTf[:, c * R:(c + 1) * R]  # [4096, 512]
        adjT_dev = np.ascontiguousarray(
            blk.reshape(NT, 128, R).transpose(1, 0, 2).reshape(128, NT * R))
        m = dict(shared)
        m["adjT"] = adjT_dev
        m["xT_loc"] = np.ascontiguousarray(xT[:, c * R:(c + 1) * R])
        in_maps.append(m)
    return in_maps


def run(inputs, trace=False):
    from concourse import bass_utils
    _install_ntff_hook()
    if "nc" not in _CACHE:
        _CACHE["nc"] = _build_nc()
    nc = _CACHE["nc"]
    in_maps = _host_prep(inputs)
    res = bass_utils.run_bass_kernel_spmd(
        nc, in_maps, core_ids=list(range(NCORES)), trace=trace)
    node_logits = np.concatenate(
        [np.ascontiguousarray(res.results[c]["logitsT"].T) for c in range(NCORES)],
        axis=0)
    h = np.concatenate([res.results[c]["out_h"] for c in range(NCORES)], axis=0)
    contagion = res.results[0]["out_con"]
    return (node_logits, h, contagion), res


def kernel(**inputs):
    (node_logits, h, contagion), _ = run(inputs, trace=False)
    return (node_logits.astype(np.float32), h.astype(np.float32),
            contagion.astype(np.float32))
